# revision 3
# baseline (speedup 1.0000x reference)
"""Trainium2 Bass kernel v3 for EnhancedMultiHeadSelfAttention.

Sharding: tensor-parallel attention by heads (core c owns heads 2c, 2c+1 over
ALL 2048 tokens) + sequence-parallel FFN (core c owns tokens 256c..256c+255).
The out-projection partial [2048, 1024] is combined with a single DRAM
ReduceScatter (layout [8, 1024, 256] so the flat-chunk scatter hands each core
its own 256 token columns, feature-major).

Math notes (beyond the baseline's):
 - LN1 is folded into the QKV matmuls: with per-token mean mu and std sd,
   W^T LN(x) = (W diag(g))^T x * r - mu*r*cg + cb  (r = 1/sd, cg = W^T g,
   cb = b + W^T ln1_b).  Dividing by r>0 is free for Q and K (cosine attention
   normalizes them per token), so q' = Wg^T x + cg*(-mu) + cb*sd — one K=2
   rank-1 matmul accumulated into the projection PSUM group. V keeps the same
   rank-1 term and a final per-token r multiply (applied post-transpose where
   tokens sit on partitions).
 - The per-token r for V reaches token-partitions via a tiny K=1 transposing
   matmul (lhsT = r row-slice, rhs = [1,1] ones).
 - softmax needs no max-subtraction; only the key-side lcc bias matters; it is
   applied MULTIPLICATIVELY (exp(s+b) = exp(s)*exp(b)) by scaling V's rows and
   the appended denominator column by host-computed exp(b_k), so the exp
   activation needs no bias and can batch two key-chunks per instruction.
 - FFN weights, h, normed2, and the ReduceScatter payload are bf16 (PSUM
   accumulation stays fp32).
"""

import ml_dtypes
import numpy as np

import concourse.bass as bass
import concourse.tile as tile
from concourse import bacc, mybir
from concourse.bass_utils import run_bass_kernel_spmd

F32 = mybir.dt.float32
F32R = mybir.dt.float32r
BF16 = mybir.dt.bfloat16

L = 2048          # sequence length
D = 1024          # model dim
H = 16            # heads (2 per core)
DH = 64           # head dim
FF = 4096         # ffn hidden
P = 128           # partitions
NCORES = 8
LQ = L // NCORES  # 256 own tokens per core (FFN + output)
DC = D // P       # 8 d-model chunks
FC = FF // P      # 32 ffn chunks
KC = L // P       # 16 key chunks of 128
NBLK = 4          # token blocks of 512
BLK = L // NBLK   # 512

GELU_FUNC = mybir.ActivationFunctionType.Gelu

LN_EPS = 1e-5
NORM_EPS = 1e-12
SCALING = DH ** -0.5
LCC = 0.1


def _mm(nc, out, lhsT, rhs, start, stop):
    assert lhsT.dtype == rhs.dtype and lhsT.dtype in (F32R, BF16), \
        (lhsT.dtype, rhs.dtype)
    nc.tensor.matmul(out, lhsT, rhs, start=start, stop=stop)


def emit(tc):
    nc = tc.nc

    xt = nc.dram_tensor("xt", [D, L], BF16, kind="ExternalInput").ap()
    xot = nc.dram_tensor("xot", [D, LQ], F32R, kind="ExternalInput").ap()
    wq2 = nc.dram_tensor("wq2", [D, P], BF16, kind="ExternalInput").ap()
    wk2 = nc.dram_tensor("wk2", [D, P], BF16, kind="ExternalInput").ap()
    wv2 = nc.dram_tensor("wv2", [D, P], BF16, kind="ExternalInput").ap()
    wo2 = nc.dram_tensor("wo2", [P, D], F32R, kind="ExternalInput").ap()
    wf1 = nc.dram_tensor("wf1", [D, FF], BF16, kind="ExternalInput").ap()
    wf2 = nc.dram_tensor("wf2", [FF, D], BF16, kind="ExternalInput").ap()
    cgbq = nc.dram_tensor("cgbq", [2, P], BF16, kind="ExternalInput").ap()
    cgbk = nc.dram_tensor("cgbk", [2, P], BF16, kind="ExternalInput").ap()
    cgbv = nc.dram_tensor("cgbv", [2, P], BF16, kind="ExternalInput").ap()
    bo = nc.dram_tensor("bo", [P, DC], F32, kind="ExternalInput").ap()
    bf1 = nc.dram_tensor("bf1", [P, FC], F32, kind="ExternalInput").ap()
    bf2 = nc.dram_tensor("bf2", [P, DC], F32, kind="ExternalInput").ap()
    explcc = nc.dram_tensor("explcc", [P, KC], F32, kind="ExternalInput").ap()
    sel2 = nc.dram_tensor("sel2", [P, 2], F32R, kind="ExternalInput").ap()
    selb2 = nc.dram_tensor("selb2", [2, P], F32R, kind="ExternalInput").ap()
    ident = nc.dram_tensor("ident", [P, P], F32R, kind="ExternalInput").ap()
    ones1r = nc.dram_tensor("ones1r", [1, P], F32R, kind="ExternalInput").ap()
    ones1f = nc.dram_tensor("ones1f", [1, P], F32, kind="ExternalInput").ap()
    onesc = nc.dram_tensor("onesc", [P, 1], F32R, kind="ExternalInput").ap()
    out_t = nc.dram_tensor("out_t", [D, LQ], F32, kind="ExternalOutput").ap()

    xt3 = xt.rearrange("(c p) t -> p c t", p=P)        # [128, 8, 2048]
    xot3 = xot.rearrange("(c p) t -> p c t", p=P)      # [128, 8, 256]
    wq3 = wq2.rearrange("(c p) n -> p c n", p=P)       # [128, 8, 128]
    wk3 = wk2.rearrange("(c p) n -> p c n", p=P)
    wv3 = wv2.rearrange("(c p) n -> p c n", p=P)
    wf13 = wf1.rearrange("(c p) n -> p c n", p=P)      # [128, 8, 4096]
    wf23 = wf2.rearrange("(c p) n -> p c n", p=P)      # [128, 32, 1024]
    out3 = out_t.rearrange("(c p) t -> p c t", p=P)    # [128, 8, 256]

    # ---- x blocks first: the stats/QKV pipeline is the critical path ----
    xb_pool0 = tc.alloc_tile_pool(name="xb", bufs=NBLK)
    xbs = []
    for b in range(NBLK):
        xb = xb_pool0.tile([P, DC, BLK], BF16, tag="xb", name=f"xb{b}")
        nc.sync.dma_start(xb, xt3[:, :, b * BLK:(b + 1) * BLK])
        xbs.append(xb)

    # ---- persistent small constants -------------------------------------
    singles = tc.alloc_tile_pool(name="singles", bufs=1)
    ones_1x128 = singles.tile([1, P], F32R)
    nc.sync.dma_start(ones_1x128, ones1r)
    ones1f_sb = singles.tile([1, P], F32)
    nc.sync.dma_start(ones1f_sb, ones1f)
    ones_col = singles.tile([P, 1], F32R)
    nc.sync.dma_start(ones_col, onesc)
    ones_col_bf = singles.tile([P, 1], BF16)
    nc.gpsimd.dma_start(ones_col_bf, onesc)
    sel2_sb = singles.tile([P, 2], F32R)
    nc.sync.dma_start(sel2_sb, sel2)
    selb2_sb = singles.tile([2, P], F32R)
    nc.sync.dma_start(selb2_sb, selb2)
    ident_sb = singles.tile([P, P], F32R)
    nc.sync.dma_start(ident_sb, ident)
    cgbq_sb = singles.tile([2, P], BF16)
    nc.sync.dma_start(cgbq_sb, cgbq)
    cgbk_sb = singles.tile([2, P], BF16)
    nc.sync.dma_start(cgbk_sb, cgbk)
    cgbv_sb = singles.tile([2, P], BF16)
    nc.sync.dma_start(cgbv_sb, cgbv)
    bo_sb = singles.tile([P, DC], F32)
    nc.sync.dma_start(bo_sb, bo)
    bf1_sb = singles.tile([P, FC], F32)
    nc.sync.dma_start(bf1_sb, bf1)
    bf2_sb = singles.tile([P, DC], F32)
    nc.sync.dma_start(bf2_sb, bf2)
    explcc_sb = singles.tile([P, KC], F32)
    nc.sync.dma_start(explcc_sb, explcc)
    wo2_sb = singles.tile([P, DC, P], F32R)
    nc.sync.dma_start(wo2_sb, wo2.rearrange("p (c n) -> p c n", n=P))
    wq_sb = singles.tile([P, DC, P], BF16)
    nc.sync.dma_start(wq_sb, wq3)
    wk_sb = singles.tile([P, DC, P], BF16)
    nc.sync.dma_start(wk_sb, wk3)
    wv_sb = singles.tile([P, DC, P], BF16)
    nc.sync.dma_start(wv_sb, wv3)
    eps_sb = singles.tile([1, 1], F32)
    nc.vector.memset(eps_sb, LN_EPS)

    # persistent activation tiles (released before FFN where possible)
    qk_pool = tc.alloc_tile_pool(name="qk", bufs=1)
    q_t = qk_pool.tile([P, L], F32R)     # [2 heads x 64 dims, tokens]
    k_t = qk_pool.tile([P, L], F32R)
    v_tm = qk_pool.tile([P, KC, 2 * (DH + 1)], F32R)  # [keys, kc, (d+1)*2h]

    # DRAM scratch: out-proj partial, one buffer per own-token half so the
    # first ReduceScatter can fire while the second half's attention runs
    LH = LQ // 2
    podram_pool = tc.alloc_tile_pool(name="podram", bufs=1, space="DRAM")
    po_dram = podram_pool.tile([2, NCORES, D, LH], BF16)
    rs_pool = tc.alloc_tile_pool(name="rsdram", bufs=1, space="DRAM")
    rs_dram = rs_pool.tile([2, D, LH], BF16)

    # =====================================================================
    # Phase 1: stats + QKV projections for the core's 2 heads, all tokens
    # =====================================================================
    with (
        tc.tile_pool(name="sq", bufs=2) as sq_pool,
        tc.tile_pool(name="smalls", bufs=2) as smalls,
        tc.tile_pool(name="vstage", bufs=2) as vstage_pool,
        tc.tile_pool(name="ps_stat", bufs=1, space="PSUM") as ps_stat,
        tc.tile_pool(name="ps_mm", bufs=3, space="PSUM") as ps_mm,
        tc.tile_pool(name="ps_nrm", bufs=1, space="PSUM") as ps_nrm,
        tc.tile_pool(name="ps_vt", bufs=1, space="PSUM") as ps_vt_pool,
    ):
        for b in range(NBLK):
            sl = slice(b * BLK, (b + 1) * BLK)
            xb = xbs[b]
            # token stats: sums (row 0) and sum-of-squares (row 1) via
            # ones-column matmuls into disjoint partition rows of one bank
            sums = ps_stat.tile([1, BLK], F32, tag="sums")
            sumsq = ps_stat.tile([1, BLK], F32, tag="sumsq")
            xsq = sq_pool.tile([P, DC, BLK], F32R, tag="xsq")
            nc.scalar.square(xsq, xb)
            for c in range(DC):
                _mm(nc, sums, ones_col_bf, xb[:, c, :], c == 0, c == DC - 1)
                _mm(nc, sumsq, ones_col, xsq[:, c, :], c == 0, c == DC - 1)
            # rhs2 = [-mu; sd] for the rank-1 LN fold; r = 1/sd for V
            mu = smalls.tile([1, BLK], F32, tag="mu")
            nc.vector.tensor_scalar_mul(mu, sums, 1.0 / D)
            ex2 = smalls.tile([1, BLK], F32, tag="ex2")
            nc.vector.tensor_scalar_mul(ex2, sumsq, 1.0 / D)
            var = smalls.tile([1, BLK], F32, tag="var")
            nc.vector.tensor_mul(var, mu, mu)
            nc.vector.tensor_sub(var, ex2, var)
            rhs2 = smalls.tile([2, BLK], BF16, tag="rhs2")
            sd0 = smalls.tile([1, BLK], F32, tag="sd0")
            nc.scalar.activation(sd0, var,
                                 func=mybir.ActivationFunctionType.Sqrt,
                                 bias=eps_sb, scale=1.0)
            with nc.allow_low_precision(reason="f32r matmul operand"):
                nc.vector.tensor_scalar_mul(rhs2[0:1, :], mu, -1.0)
            nc.gpsimd.dma_start(rhs2[1:2, :], sd0)
            r_row = smalls.tile([1, BLK], F32, tag="rrow")
            nc.vector.reciprocal(r_row, sd0)

            # Q / K with cosine normalization folded
            for (wsb, cgb, dst, scaled) in (
                (wq_sb, cgbq_sb, q_t, True),
                (wk_sb, cgbk_sb, k_t, False),
            ):
                ps = ps_mm.tile([P, BLK], F32, tag="mm")
                for c in range(DC):
                    _mm(nc, ps, wsb[:, c, :], xb[:, c, :], c == 0, False)
                _mm(nc, ps, cgb, rhs2, False, True)
                psq = sq_pool.tile([P, BLK], F32R, tag="psq")
                nc.scalar.square(psq, ps)
                nsq = ps_nrm.tile([2, BLK], F32, tag="aux")
                _mm(nc, nsq, sel2_sb, psq, True, True)
                sdq = smalls.tile([2, BLK], F32, tag="sdq")
                nc.scalar.activation(sdq, nsq,
                                     func=mybir.ActivationFunctionType.Sqrt,
                                     bias=0.0, scale=1.0)
                nc.vector.tensor_scalar_max(sdq, sdq, NORM_EPS)
                rec = smalls.tile([2, BLK], F32R, tag="rec")
                with nc.allow_low_precision(reason="f32r matmul operand"):
                    nc.vector.reciprocal(rec, sdq)
                if scaled:
                    nc.vector.tensor_scalar_mul(rec, rec, SCALING)
                rbc = ps_nrm.tile([P, BLK], F32, tag="rbc")
                _mm(nc, rbc, selb2_sb, rec, True, True)
                rbc_sb = smalls.tile([P, BLK], F32, tag="rbcsb")
                nc.vector.tensor_copy(rbc_sb, rbc)
                nc.vector.tensor_mul(dst[:, sl], ps, rbc_sb)

            # V: rank-1 fold, PE transpose to token-major, then r multiply
            ps = ps_mm.tile([P, BLK], F32, tag="mm")
            for c in range(DC):
                _mm(nc, ps, wv_sb[:, c, :], xb[:, c, :], c == 0, False)
            _mm(nc, ps, cgbv_sb, rhs2, False, True)
            vstage = vstage_pool.tile([P, BLK], F32R, tag="vstage")
            nc.vector.tensor_copy(vstage, ps)
            for t in range(4):
                kc = b * 4 + t
                tsl = slice(t * P, (t + 1) * P)
                vt = ps_vt_pool.tile([P, BLK], F32R, tag="vt")
                nc.tensor.matmul(vt[:, 0:P], vstage[:, tsl], ident_sb,
                                 is_transpose=True, start=True, stop=True)
                rtm = ps_nrm.tile([P, 1], F32, tag="aux")
                nc.tensor.matmul(rtm, r_row[:, tsl], ones1f_sb[:, 0:1],
                                 start=True, stop=True)
                # fold exp(key-side lcc bias) into V rows and the ones column
                rtme = smalls.tile([P, 1], F32, tag="rtme")
                nc.vector.tensor_mul(rtme, explcc_sb[:, kc:kc + 1], rtm)
                nc.vector.tensor_scalar_mul(v_tm[:, kc, 0:DH], vt[:, 0:DH],
                                            rtme)
                nc.vector.tensor_scalar_mul(v_tm[:, kc, DH + 1:2 * DH + 1],
                                            vt[:, DH:2 * DH], rtme)
            nc.gpsimd.tensor_copy(
                v_tm[:, b * 4:(b + 1) * 4, DH:DH + 1],
                explcc_sb[:, b * 4:(b + 1) * 4].unsqueeze(2))
            nc.gpsimd.tensor_copy(
                v_tm[:, b * 4:(b + 1) * 4, 2 * DH + 1:],
                explcc_sb[:, b * 4:(b + 1) * 4].unsqueeze(2))

    # =====================================================================
    # Phase 2: attention for 2 heads over all queries + out-proj partials
    # (first half of wf1 prefetches in the background; rest streams in ff1)
    # =====================================================================
    wf1_pool = tc.alloc_tile_pool(name="wf1sb", bufs=1)
    wf1_sb = wf1_pool.tile([P, DC, FF], BF16)
    for sl4 in range(4):
        nc.sync.dma_start(wf1_sb[:, :, sl4 * FF // 4:(sl4 + 1) * FF // 4],
                          wf13[:, :, sl4 * FF // 4:(sl4 + 1) * FF // 4])

    with (
        tc.tile_pool(name="eh", bufs=3) as eh_pool,
        tc.tile_pool(name="apair", bufs=2) as apair_pool,
        tc.tile_pool(name="rsc", bufs=2) as rsc_pool,
        tc.tile_pool(name="postage", bufs=3) as postage_pool,
        tc.tile_pool(name="ps_sc", bufs=2, space="PSUM") as ps_sc,
        tc.tile_pool(name="ps_acc", bufs=2, space="PSUM") as ps_acc,
        tc.tile_pool(name="ps_rbc", bufs=1, space="PSUM") as ps_rbc,
        tc.tile_pool(name="ps_op", bufs=1, space="PSUM") as ps_op,
    ):
        # query sets: half z of every 256-token chunk, chunk groups of 4.
        # set (h, g) covers queries {256c + h*128 + t : c in 4g..4g+3}.
        def qview(src, j, g, h):
            return src[j * DH:(j + 1) * DH, :].rearrange(
                "p (c z t) -> p c z t", z=2, t=LH)[:, 4 * g:4 * g + 4, h, :]

        for h in range(2):
            for g in range(2):
                apair = apair_pool.tile([P, BLK], F32R, tag="apair")
                for j in range(2):
                    acc = ps_acc.tile([DH + 1, BLK], F32, tag="acc")
                    for kc2 in range(KC // 2):
                        ps = ps_sc.tile([P, 2, BLK], F32, tag="sc")
                        eh = eh_pool.tile([P, 2, BLK], F32R, tag="eh")
                        for u in range(2):
                            kc = 2 * kc2 + u
                            _mm(nc, ps[:, u, :],
                                k_t[j * DH:(j + 1) * DH, kc * P:(kc + 1) * P],
                                qview(q_t, j, g, h), True, True)
                        nc.scalar.activation(
                            eh, ps, func=mybir.ActivationFunctionType.Exp,
                            bias=0.0, scale=1.0)
                        for u in range(2):
                            kc = 2 * kc2 + u
                            _mm(nc, acc,
                                v_tm[:, kc, j * (DH + 1):(j + 1) * (DH + 1)],
                                eh[:, u, :], kc == 0, kc == KC - 1)
                    recip = rsc_pool.tile([1, BLK], F32R, tag="recip")
                    with nc.allow_low_precision(reason="f32r matmul operand"):
                        nc.vector.reciprocal(recip, acc[DH:DH + 1, :])
                    rbc = ps_rbc.tile([DH, BLK], F32, tag="rbc")
                    _mm(nc, rbc, ones_1x128[:, 0:DH], recip, True, True)
                    rbc_sb = rsc_pool.tile([DH, BLK], F32, tag="rbcsb")
                    nc.vector.tensor_copy(rbc_sb, rbc)
                    nc.vector.tensor_mul(apair[j * DH:(j + 1) * DH, :],
                                         acc[0:DH, :], rbc_sb)
                # out-projection partial for this query set
                postage = postage_pool.tile([P, DC, BLK], BF16, tag="po")
                for o in range(DC):
                    pso = ps_op.tile([P, BLK], F32, tag="op")
                    _mm(nc, pso, wo2_sb[:, o, :], apair, True, True)
                    nc.vector.tensor_copy(postage[:, o, :], pso)
                for o in range(DC):
                    nc.sync.dma_start(
                        po_dram[h, 4 * g:4 * g + 4, o * P:(o + 1) * P, :]
                        .rearrange("s p t -> p s t"),
                        postage[:, o, :].rearrange("p (s t) -> p s t", t=LH))
            if h == 0:
                nc.gpsimd.collective_compute(
                    "ReduceScatter", mybir.AluOpType.add,
                    replica_groups=[list(range(NCORES))],
                    ins=[po_dram[0]], outs=[rs_dram[0]])

    # =====================================================================
    # Phase 3/4 pipelined by own-token half: residual+LN2+ff1 for half A
    # overlap the second ReduceScatter; ff2 runs monolithic at the end.
    # =====================================================================
    with (
        tc.tile_pool(name="x2p", bufs=1) as x2_pool,
        tc.tile_pool(name="ffsq", bufs=2) as ffsq_pool,
        tc.tile_pool(name="ffsm", bufs=2) as ffsm,
        tc.tile_pool(name="ht", bufs=1) as ht_pool,
        tc.tile_pool(name="wf2s", bufs=8) as wf2s,
        tc.tile_pool(name="outsb", bufs=2) as outsb_pool,
    ):
        x2 = x2_pool.tile([P, DC, LQ], F32R)
        xo2 = x2_pool.tile([P, DC, LQ], F32R)
        nc.sync.dma_start(xo2, xot3)
        normed2 = x2_pool.tile([P, DC, LQ], BF16)
        h_t = ht_pool.tile([P, FC, LQ], BF16)
        ps_mm3 = tc.alloc_tile_pool(name="ps_mm3", bufs=3, space="PSUM")

        def half_ln_ff1(h):
            hsl = slice(h * LH, (h + 1) * LH)
            rs_sb = x2_pool.tile([P, DC, LH], BF16, tag="rssb", bufs=2)
            nc.sync.dma_start(
                rs_sb, rs_dram[h].rearrange("(c p) t -> p c t", p=P))
            for o in range(DC):
                nc.vector.tensor_scalar_add(x2[:, o, hsl], rs_sb[:, o, :],
                                            bo_sb[:, o:o + 1])
                nc.vector.tensor_add(x2[:, o, hsl], x2[:, o, hsl],
                                     xo2[:, o, hsl])
            with (
                tc.tile_pool(name=f"ps_st{h}", bufs=1, space="PSUM") as ps3,
                tc.tile_pool(name=f"ps_cf{h}", bufs=2, space="PSUM") as psc3,
            ):
                sums = ps3.tile([1, LH], F32, tag="sums")
                sumsq = ps3.tile([1, LH], F32, tag="sumsq")
                for c in range(DC):
                    xsq = ffsq_pool.tile([P, LH], F32R, tag="xsq")
                    nc.scalar.square(xsq, x2[:, c, hsl])
                    _mm(nc, sums, ones_col, x2[:, c, hsl], c == 0, c == DC - 1)
                    _mm(nc, sumsq, ones_col, xsq, c == 0, c == DC - 1)
                mu = ffsm.tile([1, LH], F32, tag="mu")
                nc.vector.tensor_scalar_mul(mu, sums, 1.0 / D)
                ex2 = ffsm.tile([1, LH], F32, tag="ex2")
                nc.vector.tensor_scalar_mul(ex2, sumsq, 1.0 / D)
                var = ffsm.tile([1, LH], F32, tag="var")
                nc.vector.tensor_mul(var, mu, mu)
                nc.vector.tensor_sub(var, ex2, var)
                sd = ffsm.tile([1, LH], F32, tag="sd")
                nc.scalar.activation(sd, var,
                                     func=mybir.ActivationFunctionType.Sqrt,
                                     bias=eps_sb, scale=1.0)
                rstd = ffsm.tile([1, LH], F32R, tag="rstd")
                with nc.allow_low_precision(reason="f32r matmul operand"):
                    nc.vector.reciprocal(rstd, sd)
                shift = ffsm.tile([1, LH], F32R, tag="shift")
                nc.vector.tensor_mul(shift, mu, rstd)
                nc.vector.tensor_scalar_mul(shift, shift, -1.0)
                rstd_bc = psc3.tile([P, LH], F32, tag="coef")
                shift_bc = psc3.tile([P, LH], F32, tag="coef")
                _mm(nc, rstd_bc, ones_1x128, rstd, True, True)
                _mm(nc, shift_bc, ones_1x128, shift, True, True)
                n2h = normed2[:, :, hsl]
                rb = rstd_bc.unsqueeze(1).to_broadcast(n2h.shape)
                sb = shift_bc.unsqueeze(1).to_broadcast(n2h.shape)
                nc.vector.tensor_mul(n2h, x2[:, :, hsl], rb)
                nc.vector.tensor_add(n2h, n2h, sb)
            for f in range(FC):
                ps = ps_mm3.tile([P, LH], F32, tag="mm")
                for c in range(DC):
                    _mm(nc, ps, wf1_sb[:, c, f * P:(f + 1) * P],
                        normed2[:, c, hsl], c == 0, c == DC - 1)
                nc.scalar.activation(h_t[:, f, hsl], ps, func=GELU_FUNC,
                                     bias=bf1_sb[:, f:f + 1], scale=1.0)

        half_ln_ff1(0)
        nc.gpsimd.collective_compute(
            "ReduceScatter", mybir.AluOpType.add,
            replica_groups=[list(range(NCORES))],
            ins=[po_dram[1]], outs=[rs_dram[1]])
        half_ln_ff1(1)

        # ff2: two 4-output passes (PSUM accumulation groups are
        # bank-granular, so only 4 + ff1's 3 banks fit); wf2 for the second
        # pass prefetches during the first so pass 2 is pure PE
        ps_ff2 = tc.alloc_tile_pool(name="ps_ff2", bufs=4, space="PSUM")
        wf24 = wf23.rearrange("p c (g n) -> p c g n", g=2)  # [128,32,2,512]
        for g in range(2):
            accs = [ps_ff2.tile([P, LQ], F32, tag="ff2acc",
                                name=f"ff2acc_{g}_{i}") for i in range(4)]
            for f2 in range(FC // 2):
                wf2m = wf2s.tile([P, 2, 4 * P], BF16, tag="wf2")
                nc.sync.dma_start(wf2m, wf24[:, 2 * f2:2 * f2 + 2, g, :])
                for r in range(2):
                    f = 2 * f2 + r
                    for i in range(4):
                        _mm(nc, accs[i], wf2m[:, r, i * P:(i + 1) * P],
                            h_t[:, f, :], f == 0, f == FC - 1)
            for i in range(4):
                mcol = g * 4 + i
                osb = outsb_pool.tile([P, LQ], F32, tag="osb")
                nc.vector.tensor_scalar_add(osb, accs[i],
                                            bf2_sb[:, mcol:mcol + 1])
                nc.vector.tensor_add(osb, osb, x2[:, mcol, :])
                nc.sync.dma_start(out3[:, mcol, :], osb)
        ps_ff2.release()
        ps_mm3.release()

    wf1_pool.release()
    rs_pool.release()
    podram_pool.release()
    qk_pool.release()
    singles.release()
    xb_pool0.release()


_CACHED = None


def build():
    global _CACHED
    if _CACHED is None:
        nc = bacc.Bacc("TRN2", target_bir_lowering=False, debug=False,
                       num_devices=NCORES)
        with tile.TileContext(nc) as tc:
            emit(tc)
        nc.compile()
        _CACHED = nc
    return _CACHED


def prep_inputs(inputs):
    """Host-side preprocessing: transposes, slices, LN folds."""
    f = np.float32
    x = np.asarray(inputs["x"], f)
    lcc = np.asarray(inputs["lcc_values"], f)
    w_qkv = np.asarray(inputs["w_qkv"], f)
    b_qkv = np.asarray(inputs["b_qkv"], f)
    w_out = np.asarray(inputs["w_out"], f)
    ln1_g = np.asarray(inputs["ln1_g"], f)
    ln1_b = np.asarray(inputs["ln1_b"], f)
    ln2_g = np.asarray(inputs["ln2_g"], f)
    ln2_b = np.asarray(inputs["ln2_b"], f)
    w_ff1 = np.asarray(inputs["w_ff1"], f)
    b_ff1 = np.asarray(inputs["b_ff1"], f)

    def chunked(b):  # [D] -> [128, DC] with chunk c in column c
        return np.ascontiguousarray(b.reshape(-1, P).T)

    xt = np.ascontiguousarray(x.T).astype(ml_dtypes.bfloat16)
    sel2_m = np.zeros((P, 2), f)
    sel2_m[0:DH, 0] = 1.0
    sel2_m[DH:P, 1] = 1.0

    shared = {
        "xt": xt,
        "wf1": np.ascontiguousarray(ln2_g[:, None] * w_ff1).astype(ml_dtypes.bfloat16),
        "wf2": np.ascontiguousarray(np.asarray(inputs["w_ff2"], f)).astype(ml_dtypes.bfloat16),
        "bo": chunked(np.asarray(inputs["b_out"], f)),
        "bf1": chunked(b_ff1 + ln2_b @ w_ff1),
        "bf2": chunked(np.asarray(inputs["b_ff2"], f)),
        "explcc": np.ascontiguousarray(np.exp(lcc * (0.5 * LCC)).reshape(KC, P).T),
        "sel2": sel2_m,
        "selb2": np.ascontiguousarray(sel2_m.T),
        "ident": np.eye(P, dtype=f),
        "ones1r": np.ones((1, P), f),
        "ones1f": np.ones((1, P), f),
        "onesc": np.ones((P, 1), f),
    }
    in_maps = []
    for c in range(NCORES):
        m = dict(shared)
        csl = slice(c * P, (c + 1) * P)
        wq_s = ln1_g[:, None] * w_qkv[:, 0:D][:, csl]
        wk_s = ln1_g[:, None] * w_qkv[:, D:2 * D][:, csl]
        wv_s = ln1_g[:, None] * w_qkv[:, 2 * D:3 * D][:, csl]
        m["wq2"] = np.ascontiguousarray(wq_s).astype(ml_dtypes.bfloat16)
        m["wk2"] = np.ascontiguousarray(wk_s).astype(ml_dtypes.bfloat16)
        m["wv2"] = np.ascontiguousarray(wv_s).astype(ml_dtypes.bfloat16)
        for nm, ws, bs in (
            ("cgbq", w_qkv[:, 0:D][:, csl], b_qkv[0:D][csl]),
            ("cgbk", w_qkv[:, D:2 * D][:, csl], b_qkv[D:2 * D][csl]),
            ("cgbv", w_qkv[:, 2 * D:3 * D][:, csl], b_qkv[2 * D:3 * D][csl]),
        ):
            cg = ln1_g @ ws
            cb = bs + ln1_b @ ws
            m[nm] = np.ascontiguousarray(np.stack([cg, cb])).astype(ml_dtypes.bfloat16)
        m["wo2"] = np.ascontiguousarray(w_out[csl, :])
        m["xot"] = np.ascontiguousarray(
            np.asarray(xt[:, c * LQ:(c + 1) * LQ], np.float32))
        in_maps.append(m)
    return in_maps


def kernel(**inputs):
    nc = build()
    in_maps = prep_inputs(inputs)
    res = run_bass_kernel_spmd(nc, in_maps, core_ids=list(range(NCORES)))
    out = np.concatenate([res.results[c]["out_t"] for c in range(NCORES)], axis=1)
    return np.ascontiguousarray(out.T).astype(np.float32)


# revision 4
# speedup vs baseline: 1.0971x; 1.0971x over previous
"""Trainium2 Bass kernel v3 for EnhancedMultiHeadSelfAttention.

Sharding: tensor-parallel attention by heads (core c owns heads 2c, 2c+1 over
ALL 2048 tokens) + sequence-parallel FFN (core c owns tokens 256c..256c+255).
The out-projection partial [2048, 1024] is combined with a single DRAM
ReduceScatter (layout [8, 1024, 256] so the flat-chunk scatter hands each core
its own 256 token columns, feature-major).

Math notes (beyond the baseline's):
 - LN1 is folded into the QKV matmuls: with per-token mean mu and std sd,
   W^T LN(x) = (W diag(g))^T x * r - mu*r*cg + cb  (r = 1/sd, cg = W^T g,
   cb = b + W^T ln1_b).  Dividing by r>0 is free for Q and K (cosine attention
   normalizes them per token), so q' = Wg^T x + cg*(-mu) + cb*sd — one K=2
   rank-1 matmul accumulated into the projection PSUM group. V keeps the same
   rank-1 term and a final per-token r multiply (applied post-transpose where
   tokens sit on partitions).
 - The per-token r for V reaches token-partitions via a tiny K=1 transposing
   matmul (lhsT = r row-slice, rhs = [1,1] ones).
 - softmax needs no max-subtraction; only the key-side lcc bias matters; it is
   applied MULTIPLICATIVELY (exp(s+b) = exp(s)*exp(b)) by scaling V's rows and
   the appended denominator column by host-computed exp(b_k), so the exp
   activation needs no bias and can batch two key-chunks per instruction.
 - FFN weights, h, normed2, and the ReduceScatter payload are bf16 (PSUM
   accumulation stays fp32).
"""

import ml_dtypes
import numpy as np

import concourse.bass as bass
import concourse.tile as tile
from concourse import bacc, mybir
from concourse.bass_utils import run_bass_kernel_spmd

F32 = mybir.dt.float32
F32R = mybir.dt.float32r
BF16 = mybir.dt.bfloat16

L = 2048          # sequence length
D = 1024          # model dim
H = 16            # heads (2 per core)
DH = 64           # head dim
FF = 4096         # ffn hidden
P = 128           # partitions
NCORES = 8
LQ = L // NCORES  # 256 own tokens per core (FFN + output)
DC = D // P       # 8 d-model chunks
FC = FF // P      # 32 ffn chunks
KC = L // P       # 16 key chunks of 128
NBLK = 4          # token blocks of 512
BLK = L // NBLK   # 512

GELU_FUNC = mybir.ActivationFunctionType.Gelu

LN_EPS = 1e-5
NORM_EPS = 1e-12
SCALING = DH ** -0.5
LCC = 0.1


def _mm(nc, out, lhsT, rhs, start, stop):
    assert lhsT.dtype == rhs.dtype and lhsT.dtype in (F32R, BF16), \
        (lhsT.dtype, rhs.dtype)
    nc.tensor.matmul(out, lhsT, rhs, start=start, stop=stop)


def emit(tc):
    nc = tc.nc

    xt = nc.dram_tensor("xt", [D, L], BF16, kind="ExternalInput").ap()
    xot = nc.dram_tensor("xot", [D, LQ], F32R, kind="ExternalInput").ap()
    wq2 = nc.dram_tensor("wq2", [D, P], BF16, kind="ExternalInput").ap()
    wk2 = nc.dram_tensor("wk2", [D, P], BF16, kind="ExternalInput").ap()
    wv2 = nc.dram_tensor("wv2", [D, P], BF16, kind="ExternalInput").ap()
    wo2 = nc.dram_tensor("wo2", [P, D], F32R, kind="ExternalInput").ap()
    wf1 = nc.dram_tensor("wf1", [D, FF], BF16, kind="ExternalInput").ap()
    wf2 = nc.dram_tensor("wf2", [FF, D], BF16, kind="ExternalInput").ap()
    cgbq = nc.dram_tensor("cgbq", [2, P], BF16, kind="ExternalInput").ap()
    cgbk = nc.dram_tensor("cgbk", [2, P], BF16, kind="ExternalInput").ap()
    cgbv = nc.dram_tensor("cgbv", [2, P], BF16, kind="ExternalInput").ap()
    bo = nc.dram_tensor("bo", [P, DC], F32, kind="ExternalInput").ap()
    bf1 = nc.dram_tensor("bf1", [P, FC], F32, kind="ExternalInput").ap()
    bf2 = nc.dram_tensor("bf2", [P, DC], F32, kind="ExternalInput").ap()
    explcc = nc.dram_tensor("explcc", [P, KC], F32, kind="ExternalInput").ap()
    sel2 = nc.dram_tensor("sel2", [P, 2], F32R, kind="ExternalInput").ap()
    selb2 = nc.dram_tensor("selb2", [2, P], F32R, kind="ExternalInput").ap()
    ident = nc.dram_tensor("ident", [P, P], F32R, kind="ExternalInput").ap()
    ones1r = nc.dram_tensor("ones1r", [1, P], F32R, kind="ExternalInput").ap()
    ones1f = nc.dram_tensor("ones1f", [1, P], F32, kind="ExternalInput").ap()
    onesc = nc.dram_tensor("onesc", [P, 1], F32R, kind="ExternalInput").ap()
    out_t = nc.dram_tensor("out_t", [D, LQ], F32, kind="ExternalOutput").ap()

    xt3 = xt.rearrange("(c p) t -> p c t", p=P)        # [128, 8, 2048]
    xot3 = xot.rearrange("(c p) t -> p c t", p=P)      # [128, 8, 256]
    wq3 = wq2.rearrange("(c p) n -> p c n", p=P)       # [128, 8, 128]
    wk3 = wk2.rearrange("(c p) n -> p c n", p=P)
    wv3 = wv2.rearrange("(c p) n -> p c n", p=P)
    wf13 = wf1.rearrange("(c p) n -> p c n", p=P)      # [128, 8, 4096]
    wf23 = wf2.rearrange("(c p) n -> p c n", p=P)      # [128, 32, 1024]
    out3 = out_t.rearrange("(c p) t -> p c t", p=P)    # [128, 8, 256]

    # ---- x blocks first: the stats/QKV pipeline is the critical path ----
    xb_pool0 = tc.alloc_tile_pool(name="xb", bufs=NBLK)
    xbs = []
    for b in range(NBLK):
        xb = xb_pool0.tile([P, DC, BLK], BF16, tag="xb", name=f"xb{b}")
        nc.sync.dma_start(xb, xt3[:, :, b * BLK:(b + 1) * BLK])
        xbs.append(xb)

    # ---- persistent small constants -------------------------------------
    singles = tc.alloc_tile_pool(name="singles", bufs=1)
    ones_1x128 = singles.tile([1, P], F32R)
    nc.sync.dma_start(ones_1x128, ones1r)
    ones1f_sb = singles.tile([1, P], F32)
    nc.sync.dma_start(ones1f_sb, ones1f)
    ones_col = singles.tile([P, 1], F32R)
    nc.sync.dma_start(ones_col, onesc)
    ones_col_bf = singles.tile([P, 1], BF16)
    nc.gpsimd.dma_start(ones_col_bf, onesc)
    sel2_sb = singles.tile([P, 2], F32R)
    nc.sync.dma_start(sel2_sb, sel2)
    selb2_sb = singles.tile([2, P], F32R)
    nc.sync.dma_start(selb2_sb, selb2)
    ident_sb = singles.tile([P, P], F32R)
    nc.sync.dma_start(ident_sb, ident)
    cgbq_sb = singles.tile([2, P], BF16)
    nc.sync.dma_start(cgbq_sb, cgbq)
    cgbk_sb = singles.tile([2, P], BF16)
    nc.sync.dma_start(cgbk_sb, cgbk)
    cgbv_sb = singles.tile([2, P], BF16)
    nc.sync.dma_start(cgbv_sb, cgbv)
    bo_sb = singles.tile([P, DC], F32)
    nc.sync.dma_start(bo_sb, bo)
    bf1_sb = singles.tile([P, FC], F32)
    nc.sync.dma_start(bf1_sb, bf1)
    bf2_sb = singles.tile([P, DC], F32)
    nc.sync.dma_start(bf2_sb, bf2)
    explcc_sb = singles.tile([P, KC], F32)
    nc.sync.dma_start(explcc_sb, explcc)
    wo2_sb = singles.tile([P, DC, P], F32R)
    nc.sync.dma_start(wo2_sb, wo2.rearrange("p (c n) -> p c n", n=P))
    wq_sb = singles.tile([P, DC, P], BF16)
    nc.sync.dma_start(wq_sb, wq3)
    wk_sb = singles.tile([P, DC, P], BF16)
    nc.sync.dma_start(wk_sb, wk3)
    wv_sb = singles.tile([P, DC, P], BF16)
    nc.sync.dma_start(wv_sb, wv3)
    eps_sb = singles.tile([1, 1], F32)
    nc.vector.memset(eps_sb, LN_EPS)

    # persistent activation tiles (released before FFN where possible)
    qk_pool = tc.alloc_tile_pool(name="qk", bufs=1)
    q_t = qk_pool.tile([P, L], F32R)     # [2 heads x 64 dims, tokens]
    k_t = qk_pool.tile([P, L], F32R)
    v_tm = qk_pool.tile([P, KC, 2 * (DH + 1)], F32R)  # [keys, kc, (d+1)*2h]

    # DRAM scratch: out-proj partial, one buffer per own-token half so the
    # first ReduceScatter can fire while the second half's attention runs
    LH = LQ // 2
    podram_pool = tc.alloc_tile_pool(name="podram", bufs=1, space="DRAM")
    po_dram = podram_pool.tile([2, NCORES, D, LH], BF16)
    rs_pool = tc.alloc_tile_pool(name="rsdram", bufs=1, space="DRAM")
    rs_dram = rs_pool.tile([2, D, LH], BF16)

    # =====================================================================
    # Phase 1: stats + QKV projections for the core's 2 heads, all tokens
    # =====================================================================
    with (
        tc.tile_pool(name="sq", bufs=3) as sq_pool,
        tc.tile_pool(name="smalls", bufs=3) as smalls,
        tc.tile_pool(name="vstage", bufs=3) as vstage_pool,
        tc.tile_pool(name="ps_stat", bufs=1, space="PSUM") as ps_stat,
        tc.tile_pool(name="ps_mm", bufs=3, space="PSUM") as ps_mm,
        tc.tile_pool(name="ps_nrm", bufs=1, space="PSUM") as ps_nrm,
        tc.tile_pool(name="ps_vt", bufs=1, space="PSUM") as ps_vt_pool,
    ):
        for b in range(NBLK):
            sl = slice(b * BLK, (b + 1) * BLK)
            xb = xbs[b]
            # token stats: sums (row 0) and sum-of-squares (row 1) via
            # ones-column matmuls into disjoint partition rows of one bank
            sums = ps_stat.tile([1, BLK], F32, tag="sums")
            sumsq = ps_stat.tile([1, BLK], F32, tag="sumsq")
            xsq = sq_pool.tile([P, DC, BLK], F32R, tag="xsq")
            nc.scalar.square(xsq, xb)
            for c in range(DC):
                _mm(nc, sums, ones_col_bf, xb[:, c, :], c == 0, c == DC - 1)
                _mm(nc, sumsq, ones_col, xsq[:, c, :], c == 0, c == DC - 1)
            # rhs2 = [-mu; sd] for the rank-1 LN fold; r = 1/sd for V
            mu = smalls.tile([1, BLK], F32, tag="mu")
            nc.vector.tensor_scalar_mul(mu, sums, 1.0 / D)
            ex2 = smalls.tile([1, BLK], F32, tag="ex2")
            nc.vector.tensor_scalar_mul(ex2, sumsq, 1.0 / D)
            var = smalls.tile([1, BLK], F32, tag="var")
            nc.vector.tensor_mul(var, mu, mu)
            nc.vector.tensor_sub(var, ex2, var)
            rhs2 = smalls.tile([2, BLK], BF16, tag="rhs2")
            sd0 = smalls.tile([1, BLK], F32, tag="sd0")
            nc.scalar.activation(sd0, var,
                                 func=mybir.ActivationFunctionType.Sqrt,
                                 bias=eps_sb, scale=1.0)
            with nc.allow_low_precision(reason="f32r matmul operand"):
                nc.vector.tensor_scalar_mul(rhs2[0:1, :], mu, -1.0)
            nc.gpsimd.dma_start(rhs2[1:2, :], sd0)
            r_row = smalls.tile([1, BLK], F32, tag="rrow")
            nc.vector.reciprocal(r_row, sd0)

            # Q / K with cosine normalization folded
            for (wsb, cgb, dst, scaled) in (
                (wq_sb, cgbq_sb, q_t, True),
                (wk_sb, cgbk_sb, k_t, False),
            ):
                ps = ps_mm.tile([P, BLK], F32, tag="mm")
                for c in range(DC):
                    _mm(nc, ps, wsb[:, c, :], xb[:, c, :], c == 0, False)
                _mm(nc, ps, cgb, rhs2, False, True)
                psq = sq_pool.tile([P, BLK], F32R, tag="psq")
                nc.scalar.square(psq, ps)
                nsq = ps_nrm.tile([2, BLK], F32, tag="aux")
                _mm(nc, nsq, sel2_sb, psq, True, True)
                sdq = smalls.tile([2, BLK], F32, tag="sdq")
                nc.scalar.activation(sdq, nsq,
                                     func=mybir.ActivationFunctionType.Sqrt,
                                     bias=0.0, scale=1.0)
                nc.vector.tensor_scalar_max(sdq, sdq, NORM_EPS)
                rec = smalls.tile([2, BLK], F32R, tag="rec")
                with nc.allow_low_precision(reason="f32r matmul operand"):
                    nc.vector.reciprocal(rec, sdq)
                if scaled:
                    nc.vector.tensor_scalar_mul(rec, rec, SCALING)
                rbc = ps_nrm.tile([P, BLK], F32, tag="rbc")
                _mm(nc, rbc, selb2_sb, rec, True, True)
                rbc_sb = smalls.tile([P, BLK], F32, tag="rbcsb")
                nc.vector.tensor_copy(rbc_sb, rbc)
                nc.vector.tensor_mul(dst[:, sl], ps, rbc_sb)

            # V: rank-1 fold, PE transpose to token-major, then r multiply
            ps = ps_mm.tile([P, BLK], F32, tag="mm")
            for c in range(DC):
                _mm(nc, ps, wv_sb[:, c, :], xb[:, c, :], c == 0, False)
            _mm(nc, ps, cgbv_sb, rhs2, False, True)
            vstage = vstage_pool.tile([P, BLK], F32R, tag="vstage")
            nc.vector.tensor_copy(vstage, ps)
            for t in range(4):
                kc = b * 4 + t
                tsl = slice(t * P, (t + 1) * P)
                vt = ps_vt_pool.tile([P, BLK], F32R, tag="vt")
                nc.tensor.matmul(vt[:, 0:P], vstage[:, tsl], ident_sb,
                                 is_transpose=True, start=True, stop=True)
                rtm = ps_nrm.tile([P, 1], F32, tag="aux")
                nc.tensor.matmul(rtm, r_row[:, tsl], ones1f_sb[:, 0:1],
                                 start=True, stop=True)
                # fold exp(key-side lcc bias) into V rows and the ones column
                rtme = smalls.tile([P, 1], F32, tag="rtme")
                nc.vector.tensor_mul(rtme, explcc_sb[:, kc:kc + 1], rtm)
                nc.vector.tensor_scalar_mul(v_tm[:, kc, 0:DH], vt[:, 0:DH],
                                            rtme)
                nc.vector.tensor_scalar_mul(v_tm[:, kc, DH + 1:2 * DH + 1],
                                            vt[:, DH:2 * DH], rtme)
            nc.gpsimd.tensor_copy(
                v_tm[:, b * 4:(b + 1) * 4, DH:DH + 1],
                explcc_sb[:, b * 4:(b + 1) * 4].unsqueeze(2))
            nc.gpsimd.tensor_copy(
                v_tm[:, b * 4:(b + 1) * 4, 2 * DH + 1:],
                explcc_sb[:, b * 4:(b + 1) * 4].unsqueeze(2))

    # =====================================================================
    # Phase 2: attention for 2 heads over all queries + out-proj partials
    # (first half of wf1 prefetches in the background; rest streams in ff1)
    # =====================================================================
    wf1_pool = tc.alloc_tile_pool(name="wf1sb", bufs=1)
    wf1_sb = wf1_pool.tile([P, DC, FF], BF16)
    for sl4 in range(4):
        nc.sync.dma_start(wf1_sb[:, :, sl4 * FF // 4:(sl4 + 1) * FF // 4],
                          wf13[:, :, sl4 * FF // 4:(sl4 + 1) * FF // 4])

    with (
        tc.tile_pool(name="eh", bufs=4) as eh_pool,
        tc.tile_pool(name="apair", bufs=3) as apair_pool,
        tc.tile_pool(name="rsc", bufs=3) as rsc_pool,
        tc.tile_pool(name="postage", bufs=2) as postage_pool,
        tc.tile_pool(name="ps_sc", bufs=2, space="PSUM") as ps_sc,
        tc.tile_pool(name="ps_acc", bufs=2, space="PSUM") as ps_acc,
        tc.tile_pool(name="ps_rbc", bufs=1, space="PSUM") as ps_rbc,
        tc.tile_pool(name="ps_op", bufs=1, space="PSUM") as ps_op,
    ):
        # query sets: half z of every 256-token chunk, chunk groups of 4.
        # set (h, g) covers queries {256c + h*128 + t : c in 4g..4g+3}.
        def qview(src, j, g, h):
            return src[j * DH:(j + 1) * DH, :].rearrange(
                "p (c z t) -> p c z t", z=2, t=LH)[:, 4 * g:4 * g + 4, h, :]

        for h in range(2):
            for g in range(2):
                apair = apair_pool.tile([P, BLK], F32R, tag="apair")
                for j in range(2):
                    acc = ps_acc.tile([DH + 1, BLK], F32, tag="acc")
                    for kc2 in range(KC // 2):
                        ps = ps_sc.tile([P, 2, BLK], F32, tag="sc")
                        eh = eh_pool.tile([P, 2, BLK], F32R, tag="eh")
                        for u in range(2):
                            kc = 2 * kc2 + u
                            _mm(nc, ps[:, u, :],
                                k_t[j * DH:(j + 1) * DH, kc * P:(kc + 1) * P],
                                qview(q_t, j, g, h), True, True)
                        nc.scalar.activation(
                            eh, ps, func=mybir.ActivationFunctionType.Exp,
                            bias=0.0, scale=1.0)
                        for u in range(2):
                            kc = 2 * kc2 + u
                            _mm(nc, acc,
                                v_tm[:, kc, j * (DH + 1):(j + 1) * (DH + 1)],
                                eh[:, u, :], kc == 0, kc == KC - 1)
                    recip = rsc_pool.tile([1, BLK], F32R, tag="recip")
                    with nc.allow_low_precision(reason="f32r matmul operand"):
                        nc.vector.reciprocal(recip, acc[DH:DH + 1, :])
                    rbc = ps_rbc.tile([DH, BLK], F32, tag="rbc")
                    _mm(nc, rbc, ones_1x128[:, 0:DH], recip, True, True)
                    rbc_sb = rsc_pool.tile([DH, BLK], F32, tag="rbcsb")
                    nc.vector.tensor_copy(rbc_sb, rbc)
                    nc.vector.tensor_mul(apair[j * DH:(j + 1) * DH, :],
                                         acc[0:DH, :], rbc_sb)
                # out-projection partial for this query set
                postage = postage_pool.tile([P, DC, BLK], BF16, tag="po")
                for o in range(DC):
                    pso = ps_op.tile([P, BLK], F32, tag="op")
                    _mm(nc, pso, wo2_sb[:, o, :], apair, True, True)
                    nc.vector.tensor_copy(postage[:, o, :], pso)
                for o in range(DC):
                    nc.sync.dma_start(
                        po_dram[h, 4 * g:4 * g + 4, o * P:(o + 1) * P, :]
                        .rearrange("s p t -> p s t"),
                        postage[:, o, :].rearrange("p (s t) -> p s t", t=LH))
            if h == 0:
                nc.gpsimd.collective_compute(
                    "ReduceScatter", mybir.AluOpType.add,
                    replica_groups=[list(range(NCORES))],
                    ins=[po_dram[0]], outs=[rs_dram[0]])

    # =====================================================================
    # Phase 3/4 pipelined by own-token half: residual+LN2+ff1 for half A
    # overlap the second ReduceScatter; ff2 runs monolithic at the end.
    # =====================================================================
    with (
        tc.tile_pool(name="x2p", bufs=1) as x2_pool,
        tc.tile_pool(name="ffsq", bufs=2) as ffsq_pool,
        tc.tile_pool(name="ffsm", bufs=2) as ffsm,
        tc.tile_pool(name="ht", bufs=1) as ht_pool,
        tc.tile_pool(name="wf2s", bufs=8) as wf2s,
        tc.tile_pool(name="outsb", bufs=2) as outsb_pool,
    ):
        x2 = x2_pool.tile([P, DC, LQ], F32R)
        xo2 = x2_pool.tile([P, DC, LQ], F32R)
        nc.sync.dma_start(xo2, xot3)
        normed2 = x2_pool.tile([P, DC, LQ], BF16)
        h_t = ht_pool.tile([P, FC, LQ], BF16)
        ps_mm3 = tc.alloc_tile_pool(name="ps_mm3", bufs=4, space="PSUM")

        def half_ln_ff1(h):
            hsl = slice(h * LH, (h + 1) * LH)
            rs_sb = x2_pool.tile([P, DC, LH], BF16, tag="rssb", bufs=2)
            nc.sync.dma_start(
                rs_sb, rs_dram[h].rearrange("(c p) t -> p c t", p=P))
            for o in range(DC):
                nc.vector.tensor_scalar_add(x2[:, o, hsl], rs_sb[:, o, :],
                                            bo_sb[:, o:o + 1])
                nc.vector.tensor_add(x2[:, o, hsl], x2[:, o, hsl],
                                     xo2[:, o, hsl])
            with (
                tc.tile_pool(name=f"ps_st{h}", bufs=1, space="PSUM") as ps3,
                tc.tile_pool(name=f"ps_cf{h}", bufs=2, space="PSUM") as psc3,
            ):
                sums = ps3.tile([1, LH], F32, tag="sums")
                sumsq = ps3.tile([1, LH], F32, tag="sumsq")
                for c in range(DC):
                    xsq = ffsq_pool.tile([P, LH], F32R, tag="xsq")
                    nc.scalar.square(xsq, x2[:, c, hsl])
                    _mm(nc, sums, ones_col, x2[:, c, hsl], c == 0, c == DC - 1)
                    _mm(nc, sumsq, ones_col, xsq, c == 0, c == DC - 1)
                mu = ffsm.tile([1, LH], F32, tag="mu")
                nc.vector.tensor_scalar_mul(mu, sums, 1.0 / D)
                ex2 = ffsm.tile([1, LH], F32, tag="ex2")
                nc.vector.tensor_scalar_mul(ex2, sumsq, 1.0 / D)
                var = ffsm.tile([1, LH], F32, tag="var")
                nc.vector.tensor_mul(var, mu, mu)
                nc.vector.tensor_sub(var, ex2, var)
                sd = ffsm.tile([1, LH], F32, tag="sd")
                nc.scalar.activation(sd, var,
                                     func=mybir.ActivationFunctionType.Sqrt,
                                     bias=eps_sb, scale=1.0)
                rstd = ffsm.tile([1, LH], F32R, tag="rstd")
                with nc.allow_low_precision(reason="f32r matmul operand"):
                    nc.vector.reciprocal(rstd, sd)
                shift = ffsm.tile([1, LH], F32R, tag="shift")
                nc.vector.tensor_mul(shift, mu, rstd)
                nc.vector.tensor_scalar_mul(shift, shift, -1.0)
                rstd_bc = psc3.tile([P, LH], F32, tag="coef")
                shift_bc = psc3.tile([P, LH], F32, tag="coef")
                _mm(nc, rstd_bc, ones_1x128, rstd, True, True)
                _mm(nc, shift_bc, ones_1x128, shift, True, True)
                n2h = normed2[:, :, hsl]
                rb = rstd_bc.unsqueeze(1).to_broadcast(n2h.shape)
                sb = shift_bc.unsqueeze(1).to_broadcast(n2h.shape)
                nc.vector.tensor_mul(n2h, x2[:, :, hsl], rb)
                nc.vector.tensor_add(n2h, n2h, sb)
            for f in range(FC):
                ps = ps_mm3.tile([P, LH], F32, tag="mm")
                for c in range(DC):
                    _mm(nc, ps, wf1_sb[:, c, f * P:(f + 1) * P],
                        normed2[:, c, hsl], c == 0, c == DC - 1)
                nc.scalar.activation(h_t[:, f, hsl], ps, func=GELU_FUNC,
                                     bias=bf1_sb[:, f:f + 1], scale=1.0)

        half_ln_ff1(0)
        nc.gpsimd.collective_compute(
            "ReduceScatter", mybir.AluOpType.add,
            replica_groups=[list(range(NCORES))],
            ins=[po_dram[1]], outs=[rs_dram[1]])
        half_ln_ff1(1)

        # ff2: two 4-output passes (PSUM accumulation groups are
        # bank-granular, so only 4 + ff1's 3 banks fit); wf2 for the second
        # pass prefetches during the first so pass 2 is pure PE
        ps_ff2 = tc.alloc_tile_pool(name="ps_ff2", bufs=4, space="PSUM")
        wf24 = wf23.rearrange("p c (g n) -> p c g n", g=2)  # [128,32,2,512]
        for g in range(2):
            accs = [ps_ff2.tile([P, LQ], F32, tag="ff2acc",
                                name=f"ff2acc_{g}_{i}") for i in range(4)]
            for f2 in range(FC // 2):
                wf2m = wf2s.tile([P, 2, 4 * P], BF16, tag="wf2")
                nc.sync.dma_start(wf2m, wf24[:, 2 * f2:2 * f2 + 2, g, :])
                for r in range(2):
                    f = 2 * f2 + r
                    for i in range(4):
                        _mm(nc, accs[i], wf2m[:, r, i * P:(i + 1) * P],
                            h_t[:, f, :], f == 0, f == FC - 1)
            for i in range(4):
                mcol = g * 4 + i
                osb = outsb_pool.tile([P, LQ], F32, tag="osb")
                nc.vector.tensor_scalar_add(osb, accs[i],
                                            bf2_sb[:, mcol:mcol + 1])
                nc.vector.tensor_add(osb, osb, x2[:, mcol, :])
                nc.sync.dma_start(out3[:, mcol, :], osb)
        ps_ff2.release()
        ps_mm3.release()

    wf1_pool.release()
    rs_pool.release()
    podram_pool.release()
    qk_pool.release()
    singles.release()
    xb_pool0.release()


_CACHED = None


def build():
    global _CACHED
    if _CACHED is None:
        nc = bacc.Bacc("TRN2", target_bir_lowering=False, debug=False,
                       num_devices=NCORES)
        with tile.TileContext(nc) as tc:
            emit(tc)
        nc.compile()
        _CACHED = nc
    return _CACHED


def prep_inputs(inputs):
    """Host-side preprocessing: transposes, slices, LN folds."""
    f = np.float32
    x = np.asarray(inputs["x"], f)
    lcc = np.asarray(inputs["lcc_values"], f)
    w_qkv = np.asarray(inputs["w_qkv"], f)
    b_qkv = np.asarray(inputs["b_qkv"], f)
    w_out = np.asarray(inputs["w_out"], f)
    ln1_g = np.asarray(inputs["ln1_g"], f)
    ln1_b = np.asarray(inputs["ln1_b"], f)
    ln2_g = np.asarray(inputs["ln2_g"], f)
    ln2_b = np.asarray(inputs["ln2_b"], f)
    w_ff1 = np.asarray(inputs["w_ff1"], f)
    b_ff1 = np.asarray(inputs["b_ff1"], f)

    def chunked(b):  # [D] -> [128, DC] with chunk c in column c
        return np.ascontiguousarray(b.reshape(-1, P).T)

    xt = np.ascontiguousarray(x.T).astype(ml_dtypes.bfloat16)
    sel2_m = np.zeros((P, 2), f)
    sel2_m[0:DH, 0] = 1.0
    sel2_m[DH:P, 1] = 1.0

    shared = {
        "xt": xt,
        "wf1": np.ascontiguousarray(ln2_g[:, None] * w_ff1).astype(ml_dtypes.bfloat16),
        "wf2": np.ascontiguousarray(np.asarray(inputs["w_ff2"], f)).astype(ml_dtypes.bfloat16),
        "bo": chunked(np.asarray(inputs["b_out"], f)),
        "bf1": chunked(b_ff1 + ln2_b @ w_ff1),
        "bf2": chunked(np.asarray(inputs["b_ff2"], f)),
        "explcc": np.ascontiguousarray(np.exp(lcc * (0.5 * LCC)).reshape(KC, P).T),
        "sel2": sel2_m,
        "selb2": np.ascontiguousarray(sel2_m.T),
        "ident": np.eye(P, dtype=f),
        "ones1r": np.ones((1, P), f),
        "ones1f": np.ones((1, P), f),
        "onesc": np.ones((P, 1), f),
    }
    in_maps = []
    for c in range(NCORES):
        m = dict(shared)
        csl = slice(c * P, (c + 1) * P)
        wq_s = ln1_g[:, None] * w_qkv[:, 0:D][:, csl]
        wk_s = ln1_g[:, None] * w_qkv[:, D:2 * D][:, csl]
        wv_s = ln1_g[:, None] * w_qkv[:, 2 * D:3 * D][:, csl]
        m["wq2"] = np.ascontiguousarray(wq_s).astype(ml_dtypes.bfloat16)
        m["wk2"] = np.ascontiguousarray(wk_s).astype(ml_dtypes.bfloat16)
        m["wv2"] = np.ascontiguousarray(wv_s).astype(ml_dtypes.bfloat16)
        for nm, ws, bs in (
            ("cgbq", w_qkv[:, 0:D][:, csl], b_qkv[0:D][csl]),
            ("cgbk", w_qkv[:, D:2 * D][:, csl], b_qkv[D:2 * D][csl]),
            ("cgbv", w_qkv[:, 2 * D:3 * D][:, csl], b_qkv[2 * D:3 * D][csl]),
        ):
            cg = ln1_g @ ws
            cb = bs + ln1_b @ ws
            m[nm] = np.ascontiguousarray(np.stack([cg, cb])).astype(ml_dtypes.bfloat16)
        m["wo2"] = np.ascontiguousarray(w_out[csl, :])
        m["xot"] = np.ascontiguousarray(
            np.asarray(xt[:, c * LQ:(c + 1) * LQ], np.float32))
        in_maps.append(m)
    return in_maps


def kernel(**inputs):
    nc = build()
    in_maps = prep_inputs(inputs)
    res = run_bass_kernel_spmd(nc, in_maps, core_ids=list(range(NCORES)))
    out = np.concatenate([res.results[c]["out_t"] for c in range(NCORES)], axis=1)
    return np.ascontiguousarray(out.T).astype(np.float32)


# revision 5
# speedup vs baseline: 1.1097x; 1.0114x over previous
"""Trainium2 Bass kernel v3 for EnhancedMultiHeadSelfAttention.

Sharding: tensor-parallel attention by heads (core c owns heads 2c, 2c+1 over
ALL 2048 tokens) + sequence-parallel FFN (core c owns tokens 256c..256c+255).
The out-projection partial [2048, 1024] is combined with a single DRAM
ReduceScatter (layout [8, 1024, 256] so the flat-chunk scatter hands each core
its own 256 token columns, feature-major).

Math notes (beyond the baseline's):
 - LN1 is folded into the QKV matmuls: with per-token mean mu and std sd,
   W^T LN(x) = (W diag(g))^T x * r - mu*r*cg + cb  (r = 1/sd, cg = W^T g,
   cb = b + W^T ln1_b).  Dividing by r>0 is free for Q and K (cosine attention
   normalizes them per token), so q' = Wg^T x + cg*(-mu) + cb*sd — one K=2
   rank-1 matmul accumulated into the projection PSUM group. V keeps the same
   rank-1 term and a final per-token r multiply (applied post-transpose where
   tokens sit on partitions).
 - The per-token r for V reaches token-partitions via a tiny K=1 transposing
   matmul (lhsT = r row-slice, rhs = [1,1] ones).
 - softmax needs no max-subtraction; only the key-side lcc bias matters; it is
   applied MULTIPLICATIVELY (exp(s+b) = exp(s)*exp(b)) by scaling V's rows and
   the appended denominator column by host-computed exp(b_k), so the exp
   activation needs no bias and can batch two key-chunks per instruction.
 - FFN weights, h, normed2, and the ReduceScatter payload are bf16 (PSUM
   accumulation stays fp32).
"""

import ml_dtypes
import numpy as np

import concourse.bass as bass
import concourse.tile as tile
from concourse import bacc, mybir
from concourse.bass_utils import run_bass_kernel_spmd

F32 = mybir.dt.float32
F32R = mybir.dt.float32r
BF16 = mybir.dt.bfloat16

L = 2048          # sequence length
D = 1024          # model dim
H = 16            # heads (2 per core)
DH = 64           # head dim
FF = 4096         # ffn hidden
P = 128           # partitions
NCORES = 8
LQ = L // NCORES  # 256 own tokens per core (FFN + output)
DC = D // P       # 8 d-model chunks
FC = FF // P      # 32 ffn chunks
KC = L // P       # 16 key chunks of 128
NBLK = 4          # token blocks of 512
BLK = L // NBLK   # 512

GELU_FUNC = mybir.ActivationFunctionType.Gelu

LN_EPS = 1e-5
NORM_EPS = 1e-12
SCALING = DH ** -0.5
LCC = 0.1


def _mm(nc, out, lhsT, rhs, start, stop):
    assert lhsT.dtype == rhs.dtype and lhsT.dtype in (F32R, BF16), \
        (lhsT.dtype, rhs.dtype)
    nc.tensor.matmul(out, lhsT, rhs, start=start, stop=stop)


def emit(tc):
    nc = tc.nc

    xt = nc.dram_tensor("xt", [D, L], BF16, kind="ExternalInput").ap()
    xot = nc.dram_tensor("xot", [D, LQ], F32R, kind="ExternalInput").ap()
    wq2 = nc.dram_tensor("wq2", [D, P], BF16, kind="ExternalInput").ap()
    wk2 = nc.dram_tensor("wk2", [D, P], BF16, kind="ExternalInput").ap()
    wv2 = nc.dram_tensor("wv2", [D, P], BF16, kind="ExternalInput").ap()
    wo2 = nc.dram_tensor("wo2", [P, D], F32R, kind="ExternalInput").ap()
    wf1 = nc.dram_tensor("wf1", [D, FF], BF16, kind="ExternalInput").ap()
    wf2 = nc.dram_tensor("wf2", [FF, D], BF16, kind="ExternalInput").ap()
    cgbq = nc.dram_tensor("cgbq", [2, P], BF16, kind="ExternalInput").ap()
    cgbk = nc.dram_tensor("cgbk", [2, P], BF16, kind="ExternalInput").ap()
    cgbv = nc.dram_tensor("cgbv", [2, P], BF16, kind="ExternalInput").ap()
    bo = nc.dram_tensor("bo", [P, DC], F32, kind="ExternalInput").ap()
    bf1 = nc.dram_tensor("bf1", [P, FC], F32, kind="ExternalInput").ap()
    bf2 = nc.dram_tensor("bf2", [P, DC], F32, kind="ExternalInput").ap()
    explcc = nc.dram_tensor("explcc", [P, KC], F32, kind="ExternalInput").ap()
    sel2 = nc.dram_tensor("sel2", [P, 2], F32R, kind="ExternalInput").ap()
    selb2 = nc.dram_tensor("selb2", [2, P], F32R, kind="ExternalInput").ap()
    selb2q = nc.dram_tensor("selb2q", [2, P], F32R, kind="ExternalInput").ap()
    ident = nc.dram_tensor("ident", [P, P], F32R, kind="ExternalInput").ap()
    ones1r = nc.dram_tensor("ones1r", [1, P], F32R, kind="ExternalInput").ap()
    ones1f = nc.dram_tensor("ones1f", [1, P], F32, kind="ExternalInput").ap()
    onesc = nc.dram_tensor("onesc", [P, 1], F32R, kind="ExternalInput").ap()
    out_t = nc.dram_tensor("out_t", [D, LQ], F32, kind="ExternalOutput").ap()

    xt3 = xt.rearrange("(c p) t -> p c t", p=P)        # [128, 8, 2048]
    xot3 = xot.rearrange("(c p) t -> p c t", p=P)      # [128, 8, 256]
    wq3 = wq2.rearrange("(c p) n -> p c n", p=P)       # [128, 8, 128]
    wk3 = wk2.rearrange("(c p) n -> p c n", p=P)
    wv3 = wv2.rearrange("(c p) n -> p c n", p=P)
    wf13 = wf1.rearrange("(c p) n -> p c n", p=P)      # [128, 8, 4096]
    wf23 = wf2.rearrange("(c p) n -> p c n", p=P)      # [128, 32, 1024]
    out3 = out_t.rearrange("(c p) t -> p c t", p=P)    # [128, 8, 256]

    # ---- x blocks first: the stats/QKV pipeline is the critical path ----
    xb_pool0 = tc.alloc_tile_pool(name="xb", bufs=NBLK)
    xbs = []
    for b in range(NBLK):
        xb = xb_pool0.tile([P, DC, BLK], BF16, tag="xb", name=f"xb{b}")
        nc.sync.dma_start(xb, xt3[:, :, b * BLK:(b + 1) * BLK])
        xbs.append(xb)

    # ---- persistent small constants -------------------------------------
    singles = tc.alloc_tile_pool(name="singles", bufs=1)
    ones_1x128 = singles.tile([1, P], F32R)
    nc.sync.dma_start(ones_1x128, ones1r)
    ones1f_sb = singles.tile([1, P], F32)
    nc.sync.dma_start(ones1f_sb, ones1f)
    ones_col = singles.tile([P, 1], F32R)
    nc.sync.dma_start(ones_col, onesc)
    ones_col_bf = singles.tile([P, 1], BF16)
    nc.gpsimd.dma_start(ones_col_bf, onesc)
    sel2_sb = singles.tile([P, 2], F32R)
    nc.sync.dma_start(sel2_sb, sel2)
    selb2_sb = singles.tile([2, P], F32R)
    nc.sync.dma_start(selb2_sb, selb2)
    selb2q_sb = singles.tile([2, P], F32R)
    nc.sync.dma_start(selb2q_sb, selb2q)
    ident_sb = singles.tile([P, P], F32R)
    nc.sync.dma_start(ident_sb, ident)
    cgbq_sb = singles.tile([2, P], BF16)
    nc.sync.dma_start(cgbq_sb, cgbq)
    cgbk_sb = singles.tile([2, P], BF16)
    nc.sync.dma_start(cgbk_sb, cgbk)
    cgbv_sb = singles.tile([2, P], BF16)
    nc.sync.dma_start(cgbv_sb, cgbv)
    bo_sb = singles.tile([P, DC], F32)
    nc.sync.dma_start(bo_sb, bo)
    bf1_sb = singles.tile([P, FC], F32)
    nc.sync.dma_start(bf1_sb, bf1)
    bf2_sb = singles.tile([P, DC], F32)
    nc.sync.dma_start(bf2_sb, bf2)
    explcc_sb = singles.tile([P, KC], F32)
    nc.sync.dma_start(explcc_sb, explcc)
    wo2_sb = singles.tile([P, DC, P], F32R)
    nc.sync.dma_start(wo2_sb, wo2.rearrange("p (c n) -> p c n", n=P))
    wq_sb = singles.tile([P, DC, P], BF16)
    nc.sync.dma_start(wq_sb, wq3)
    wk_sb = singles.tile([P, DC, P], BF16)
    nc.sync.dma_start(wk_sb, wk3)
    wv_sb = singles.tile([P, DC, P], BF16)
    nc.sync.dma_start(wv_sb, wv3)
    eps_sb = singles.tile([1, 1], F32)
    nc.vector.memset(eps_sb, LN_EPS)

    # persistent activation tiles (released before FFN where possible)
    qk_pool = tc.alloc_tile_pool(name="qk", bufs=1)
    q_t = qk_pool.tile([P, L], F32R)     # [2 heads x 64 dims, tokens]
    k_t = qk_pool.tile([P, L], F32R)
    v_tm = qk_pool.tile([P, KC, 2 * (DH + 1)], F32R)  # [keys, kc, (d+1)*2h]

    # DRAM scratch: out-proj partial, one buffer per own-token half so the
    # first ReduceScatter can fire while the second half's attention runs
    LH = LQ // 2
    podram_pool = tc.alloc_tile_pool(name="podram", bufs=1, space="DRAM")
    po_dram = podram_pool.tile([2, NCORES, D, LH], BF16)
    rs_pool = tc.alloc_tile_pool(name="rsdram", bufs=1, space="DRAM")
    rs_dram = rs_pool.tile([2, D, LH], BF16)

    # =====================================================================
    # Phase 1: stats + QKV projections for the core's 2 heads, all tokens
    # =====================================================================
    with (
        tc.tile_pool(name="sq", bufs=3) as sq_pool,
        tc.tile_pool(name="smalls", bufs=3) as smalls,
        tc.tile_pool(name="vstage", bufs=3) as vstage_pool,
        tc.tile_pool(name="ps_stat", bufs=1, space="PSUM") as ps_stat,
        tc.tile_pool(name="ps_mm", bufs=3, space="PSUM") as ps_mm,
        tc.tile_pool(name="ps_nrm", bufs=1, space="PSUM") as ps_nrm,
        tc.tile_pool(name="ps_vt", bufs=1, space="PSUM") as ps_vt_pool,
    ):
        for b in range(NBLK):
            sl = slice(b * BLK, (b + 1) * BLK)
            xb = xbs[b]
            # token stats: sums (row 0) and sum-of-squares (row 1) via
            # ones-column matmuls into disjoint partition rows of one bank
            sums = ps_stat.tile([1, BLK], F32, tag="sums")
            sumsq = ps_stat.tile([1, BLK], F32, tag="sumsq")
            xsq = sq_pool.tile([P, DC, BLK], F32R, tag="xsq")
            nc.scalar.square(xsq, xb)
            for c in range(DC):
                _mm(nc, sums, ones_col_bf, xb[:, c, :], c == 0, c == DC - 1)
                _mm(nc, sumsq, ones_col, xsq[:, c, :], c == 0, c == DC - 1)
            # rhs2 = [-mu; sd] for the rank-1 LN fold; r = 1/sd for V
            mu = smalls.tile([1, BLK], F32, tag="mu")
            nc.vector.tensor_scalar_mul(mu, sums, 1.0 / D)
            ex2 = smalls.tile([1, BLK], F32, tag="ex2")
            nc.vector.tensor_scalar_mul(ex2, sumsq, 1.0 / D)
            var = smalls.tile([1, BLK], F32, tag="var")
            nc.vector.tensor_mul(var, mu, mu)
            nc.vector.tensor_sub(var, ex2, var)
            rhs2 = smalls.tile([2, BLK], BF16, tag="rhs2")
            sd0 = smalls.tile([1, BLK], F32, tag="sd0")
            nc.scalar.activation(sd0, var,
                                 func=mybir.ActivationFunctionType.Sqrt,
                                 bias=eps_sb, scale=1.0)
            with nc.allow_low_precision(reason="f32r matmul operand"):
                nc.vector.tensor_scalar_mul(rhs2[0:1, :], mu, -1.0)
            nc.gpsimd.dma_start(rhs2[1:2, :], sd0)
            r_row = smalls.tile([1, BLK], F32, tag="rrow")
            nc.vector.reciprocal(r_row, sd0)

            # Q / K with cosine normalization folded
            for (wsb, cgb, dst, scaled) in (
                (wq_sb, cgbq_sb, q_t, True),
                (wk_sb, cgbk_sb, k_t, False),
            ):
                ps = ps_mm.tile([P, BLK], F32, tag="mm")
                for c in range(DC):
                    _mm(nc, ps, wsb[:, c, :], xb[:, c, :], c == 0, False)
                _mm(nc, ps, cgb, rhs2, False, True)
                psq = sq_pool.tile([P, BLK], F32R, tag="psq")
                nc.scalar.square(psq, ps)
                nsq = ps_nrm.tile([2, BLK], F32, tag="aux")
                _mm(nc, nsq, sel2_sb, psq, True, True)
                sdq = smalls.tile([2, BLK], F32, tag="sdq")
                nc.scalar.activation(sdq, nsq,
                                     func=mybir.ActivationFunctionType.Sqrt,
                                     bias=0.0, scale=1.0)
                rec = smalls.tile([2, BLK], F32R, tag="rec")
                with nc.allow_low_precision(reason="f32r matmul operand"):
                    nc.vector.reciprocal(rec, sdq)
                rbc = ps_nrm.tile([P, BLK], F32, tag="rbc")
                _mm(nc, rbc, selb2q_sb if scaled else selb2_sb, rec,
                    True, True)
                rbc_sb = smalls.tile([P, BLK], F32, tag="rbcsb")
                nc.vector.tensor_copy(rbc_sb, rbc)
                nc.vector.tensor_mul(dst[:, sl], ps, rbc_sb)

            # V: rank-1 fold, PE transpose to token-major, then r multiply
            ps = ps_mm.tile([P, BLK], F32, tag="mm")
            for c in range(DC):
                _mm(nc, ps, wv_sb[:, c, :], xb[:, c, :], c == 0, False)
            _mm(nc, ps, cgbv_sb, rhs2, False, True)
            vstage = vstage_pool.tile([P, BLK], F32R, tag="vstage")
            nc.vector.tensor_copy(vstage, ps)
            for t in range(4):
                kc = b * 4 + t
                tsl = slice(t * P, (t + 1) * P)
                vt = ps_vt_pool.tile([P, BLK], F32R, tag="vt")
                nc.tensor.matmul(vt[:, 0:P], vstage[:, tsl], ident_sb,
                                 is_transpose=True, start=True, stop=True)
                rtm = ps_nrm.tile([P, 1], F32, tag="aux")
                nc.tensor.matmul(rtm, r_row[:, tsl], ones1f_sb[:, 0:1],
                                 start=True, stop=True)
                # fold exp(key-side lcc bias) into V rows and the ones column
                rtme = smalls.tile([P, 1], F32, tag="rtme")
                nc.vector.tensor_mul(rtme, explcc_sb[:, kc:kc + 1], rtm)
                nc.vector.tensor_scalar_mul(v_tm[:, kc, 0:DH], vt[:, 0:DH],
                                            rtme)
                nc.vector.tensor_scalar_mul(v_tm[:, kc, DH + 1:2 * DH + 1],
                                            vt[:, DH:2 * DH], rtme)
            nc.gpsimd.tensor_copy(
                v_tm[:, b * 4:(b + 1) * 4, DH:DH + 1],
                explcc_sb[:, b * 4:(b + 1) * 4].unsqueeze(2))
            nc.gpsimd.tensor_copy(
                v_tm[:, b * 4:(b + 1) * 4, 2 * DH + 1:],
                explcc_sb[:, b * 4:(b + 1) * 4].unsqueeze(2))

    # =====================================================================
    # Phase 2: attention for 2 heads over all queries + out-proj partials
    # (first half of wf1 prefetches in the background; rest streams in ff1)
    # =====================================================================
    wf1_pool = tc.alloc_tile_pool(name="wf1sb", bufs=1)
    wf1_sb = wf1_pool.tile([P, DC, FF], BF16)
    for sl4 in range(4):
        nc.sync.dma_start(wf1_sb[:, :, sl4 * FF // 4:(sl4 + 1) * FF // 4],
                          wf13[:, :, sl4 * FF // 4:(sl4 + 1) * FF // 4])

    with (
        tc.tile_pool(name="eh", bufs=4) as eh_pool,
        tc.tile_pool(name="apair", bufs=3) as apair_pool,
        tc.tile_pool(name="rsc", bufs=3) as rsc_pool,
        tc.tile_pool(name="postage", bufs=2) as postage_pool,
        tc.tile_pool(name="ps_sc", bufs=2, space="PSUM") as ps_sc,
        tc.tile_pool(name="ps_acc", bufs=2, space="PSUM") as ps_acc,
        tc.tile_pool(name="ps_rbc", bufs=1, space="PSUM") as ps_rbc,
        tc.tile_pool(name="ps_op", bufs=1, space="PSUM") as ps_op,
    ):
        # query sets: half z of every 256-token chunk, chunk groups of 4.
        # set (h, g) covers queries {256c + h*128 + t : c in 4g..4g+3}.
        def qview(src, j, g, h):
            return src[j * DH:(j + 1) * DH, :].rearrange(
                "p (c z t) -> p c z t", z=2, t=LH)[:, 4 * g:4 * g + 4, h, :]

        for h in range(2):
            for g in range(2):
                apair = apair_pool.tile([P, BLK], F32R, tag="apair")
                for j in range(2):
                    acc = ps_acc.tile([DH + 1, BLK], F32, tag="acc")
                    for kc2 in range(KC // 2):
                        ps = ps_sc.tile([P, 2, BLK], F32, tag="sc")
                        eh = eh_pool.tile([P, 2, BLK], F32R, tag="eh")
                        for u in range(2):
                            kc = 2 * kc2 + u
                            _mm(nc, ps[:, u, :],
                                k_t[j * DH:(j + 1) * DH, kc * P:(kc + 1) * P],
                                qview(q_t, j, g, h), True, True)
                        nc.scalar.activation(
                            eh, ps, func=mybir.ActivationFunctionType.Exp,
                            bias=0.0, scale=1.0)
                        for u in range(2):
                            kc = 2 * kc2 + u
                            _mm(nc, acc,
                                v_tm[:, kc, j * (DH + 1):(j + 1) * (DH + 1)],
                                eh[:, u, :], kc == 0, kc == KC - 1)
                    recip = rsc_pool.tile([1, BLK], F32R, tag="recip")
                    with nc.allow_low_precision(reason="f32r matmul operand"):
                        nc.vector.reciprocal(recip, acc[DH:DH + 1, :])
                    rbc = ps_rbc.tile([DH, BLK], F32, tag="rbc")
                    _mm(nc, rbc, ones_1x128[:, 0:DH], recip, True, True)
                    rbc_sb = rsc_pool.tile([DH, BLK], F32, tag="rbcsb")
                    nc.vector.tensor_copy(rbc_sb, rbc)
                    nc.vector.tensor_mul(apair[j * DH:(j + 1) * DH, :],
                                         acc[0:DH, :], rbc_sb)
                # out-projection partial for this query set
                postage = postage_pool.tile([P, DC, BLK], BF16, tag="po")
                for o in range(DC):
                    pso = ps_op.tile([P, BLK], F32, tag="op")
                    _mm(nc, pso, wo2_sb[:, o, :], apair, True, True)
                    nc.vector.tensor_copy(postage[:, o, :], pso)
                for o in range(DC):
                    nc.sync.dma_start(
                        po_dram[h, 4 * g:4 * g + 4, o * P:(o + 1) * P, :]
                        .rearrange("s p t -> p s t"),
                        postage[:, o, :].rearrange("p (s t) -> p s t", t=LH))
            if h == 0:
                nc.gpsimd.collective_compute(
                    "ReduceScatter", mybir.AluOpType.add,
                    replica_groups=[list(range(NCORES))],
                    ins=[po_dram[0]], outs=[rs_dram[0]])

    # =====================================================================
    # Phase 3/4 pipelined by own-token half: residual+LN2+ff1 for half A
    # overlap the second ReduceScatter; ff2 runs monolithic at the end.
    # =====================================================================
    with (
        tc.tile_pool(name="x2p", bufs=1) as x2_pool,
        tc.tile_pool(name="ffsq", bufs=2) as ffsq_pool,
        tc.tile_pool(name="ffsm", bufs=2) as ffsm,
        tc.tile_pool(name="ht", bufs=1) as ht_pool,
        tc.tile_pool(name="wf2s", bufs=8) as wf2s,
        tc.tile_pool(name="outsb", bufs=2) as outsb_pool,
    ):
        x2 = x2_pool.tile([P, DC, LQ], F32R)
        xo2 = x2_pool.tile([P, DC, LQ], F32R)
        nc.sync.dma_start(xo2, xot3)
        normed2 = x2_pool.tile([P, DC, LQ], BF16)
        h_t = ht_pool.tile([P, FC, LQ], BF16)
        ps_mm3 = tc.alloc_tile_pool(name="ps_mm3", bufs=4, space="PSUM")

        def half_ln_ff1(h):
            hsl = slice(h * LH, (h + 1) * LH)
            rs_sb = x2_pool.tile([P, DC, LH], BF16, tag="rssb", bufs=2)
            nc.sync.dma_start(
                rs_sb, rs_dram[h].rearrange("(c p) t -> p c t", p=P))
            for o in range(DC):
                nc.vector.tensor_scalar_add(x2[:, o, hsl], rs_sb[:, o, :],
                                            bo_sb[:, o:o + 1])
                nc.vector.tensor_add(x2[:, o, hsl], x2[:, o, hsl],
                                     xo2[:, o, hsl])
            with (
                tc.tile_pool(name=f"ps_st{h}", bufs=1, space="PSUM") as ps3,
                tc.tile_pool(name=f"ps_cf{h}", bufs=2, space="PSUM") as psc3,
            ):
                sums = ps3.tile([1, LH], F32, tag="sums")
                sumsq = ps3.tile([1, LH], F32, tag="sumsq")
                for c in range(DC):
                    xsq = ffsq_pool.tile([P, LH], F32R, tag="xsq")
                    nc.scalar.square(xsq, x2[:, c, hsl])
                    _mm(nc, sums, ones_col, x2[:, c, hsl], c == 0, c == DC - 1)
                    _mm(nc, sumsq, ones_col, xsq, c == 0, c == DC - 1)
                mu = ffsm.tile([1, LH], F32, tag="mu")
                nc.vector.tensor_scalar_mul(mu, sums, 1.0 / D)
                ex2 = ffsm.tile([1, LH], F32, tag="ex2")
                nc.vector.tensor_scalar_mul(ex2, sumsq, 1.0 / D)
                var = ffsm.tile([1, LH], F32, tag="var")
                nc.vector.tensor_mul(var, mu, mu)
                nc.vector.tensor_sub(var, ex2, var)
                sd = ffsm.tile([1, LH], F32, tag="sd")
                nc.scalar.activation(sd, var,
                                     func=mybir.ActivationFunctionType.Sqrt,
                                     bias=eps_sb, scale=1.0)
                rstd = ffsm.tile([1, LH], F32R, tag="rstd")
                with nc.allow_low_precision(reason="f32r matmul operand"):
                    nc.vector.reciprocal(rstd, sd)
                shift = ffsm.tile([1, LH], F32R, tag="shift")
                nc.vector.tensor_mul(shift, mu, rstd)
                nc.vector.tensor_scalar_mul(shift, shift, -1.0)
                rstd_bc = psc3.tile([P, LH], F32, tag="coef")
                shift_bc = psc3.tile([P, LH], F32, tag="coef")
                _mm(nc, rstd_bc, ones_1x128, rstd, True, True)
                _mm(nc, shift_bc, ones_1x128, shift, True, True)
                n2h = normed2[:, :, hsl]
                rb = rstd_bc.unsqueeze(1).to_broadcast(n2h.shape)
                sb = shift_bc.unsqueeze(1).to_broadcast(n2h.shape)
                nc.vector.tensor_mul(n2h, x2[:, :, hsl], rb)
                nc.vector.tensor_add(n2h, n2h, sb)
            for f in range(FC):
                ps = ps_mm3.tile([P, LH], F32, tag="mm")
                for c in range(DC):
                    _mm(nc, ps, wf1_sb[:, c, f * P:(f + 1) * P],
                        normed2[:, c, hsl], c == 0, c == DC - 1)
                nc.scalar.activation(h_t[:, f, hsl], ps, func=GELU_FUNC,
                                     bias=bf1_sb[:, f:f + 1], scale=1.0)

        nc.gpsimd.collective_compute(
            "ReduceScatter", mybir.AluOpType.add,
            replica_groups=[list(range(NCORES))],
            ins=[po_dram[1]], outs=[rs_dram[1]])
        half_ln_ff1(0)
        half_ln_ff1(1)

        # ff2: two 4-output passes (PSUM accumulation groups are
        # bank-granular, so only 4 + ff1's 3 banks fit); wf2 for the second
        # pass prefetches during the first so pass 2 is pure PE
        ps_ff2 = tc.alloc_tile_pool(name="ps_ff2", bufs=4, space="PSUM")
        wf24 = wf23.rearrange("p c (g n) -> p c g n", g=2)  # [128,32,2,512]
        for g in range(2):
            accs = [ps_ff2.tile([P, LQ], F32, tag="ff2acc",
                                name=f"ff2acc_{g}_{i}") for i in range(4)]
            for f2 in range(FC // 2):
                wf2m = wf2s.tile([P, 2, 4 * P], BF16, tag="wf2")
                nc.sync.dma_start(wf2m, wf24[:, 2 * f2:2 * f2 + 2, g, :])
                for r in range(2):
                    f = 2 * f2 + r
                    for i in range(4):
                        _mm(nc, accs[i], wf2m[:, r, i * P:(i + 1) * P],
                            h_t[:, f, :], f == 0, f == FC - 1)
            for i in range(4):
                mcol = g * 4 + i
                osb = outsb_pool.tile([P, LQ], F32, tag="osb")
                nc.vector.tensor_scalar_add(osb, accs[i],
                                            bf2_sb[:, mcol:mcol + 1])
                nc.vector.tensor_add(osb, osb, x2[:, mcol, :])
                nc.sync.dma_start(out3[:, mcol, :], osb)
        ps_ff2.release()
        ps_mm3.release()

    wf1_pool.release()
    rs_pool.release()
    podram_pool.release()
    qk_pool.release()
    singles.release()
    xb_pool0.release()


_CACHED = None


def build():
    global _CACHED
    if _CACHED is None:
        nc = bacc.Bacc("TRN2", target_bir_lowering=False, debug=False,
                       num_devices=NCORES)
        with tile.TileContext(nc) as tc:
            emit(tc)
        nc.compile()
        _CACHED = nc
    return _CACHED


def prep_inputs(inputs):
    """Host-side preprocessing: transposes, slices, LN folds."""
    f = np.float32
    x = np.asarray(inputs["x"], f)
    lcc = np.asarray(inputs["lcc_values"], f)
    w_qkv = np.asarray(inputs["w_qkv"], f)
    b_qkv = np.asarray(inputs["b_qkv"], f)
    w_out = np.asarray(inputs["w_out"], f)
    ln1_g = np.asarray(inputs["ln1_g"], f)
    ln1_b = np.asarray(inputs["ln1_b"], f)
    ln2_g = np.asarray(inputs["ln2_g"], f)
    ln2_b = np.asarray(inputs["ln2_b"], f)
    w_ff1 = np.asarray(inputs["w_ff1"], f)
    b_ff1 = np.asarray(inputs["b_ff1"], f)

    def chunked(b):  # [D] -> [128, DC] with chunk c in column c
        return np.ascontiguousarray(b.reshape(-1, P).T)

    xt = np.ascontiguousarray(x.T).astype(ml_dtypes.bfloat16)
    sel2_m = np.zeros((P, 2), f)
    sel2_m[0:DH, 0] = 1.0
    sel2_m[DH:P, 1] = 1.0

    shared = {
        "xt": xt,
        "wf1": np.ascontiguousarray(ln2_g[:, None] * w_ff1).astype(ml_dtypes.bfloat16),
        "wf2": np.ascontiguousarray(np.asarray(inputs["w_ff2"], f)).astype(ml_dtypes.bfloat16),
        "bo": chunked(np.asarray(inputs["b_out"], f)),
        "bf1": chunked(b_ff1 + ln2_b @ w_ff1),
        "bf2": chunked(np.asarray(inputs["b_ff2"], f)),
        "explcc": np.ascontiguousarray(np.exp(lcc * (0.5 * LCC)).reshape(KC, P).T),
        "sel2": sel2_m,
        "selb2": np.ascontiguousarray(sel2_m.T),
        "selb2q": np.ascontiguousarray(sel2_m.T * (DH ** -0.5)),
        "ident": np.eye(P, dtype=f),
        "ones1r": np.ones((1, P), f),
        "ones1f": np.ones((1, P), f),
        "onesc": np.ones((P, 1), f),
    }
    in_maps = []
    for c in range(NCORES):
        m = dict(shared)
        csl = slice(c * P, (c + 1) * P)
        wq_s = ln1_g[:, None] * w_qkv[:, 0:D][:, csl]
        wk_s = ln1_g[:, None] * w_qkv[:, D:2 * D][:, csl]
        wv_s = ln1_g[:, None] * w_qkv[:, 2 * D:3 * D][:, csl]
        m["wq2"] = np.ascontiguousarray(wq_s).astype(ml_dtypes.bfloat16)
        m["wk2"] = np.ascontiguousarray(wk_s).astype(ml_dtypes.bfloat16)
        m["wv2"] = np.ascontiguousarray(wv_s).astype(ml_dtypes.bfloat16)
        for nm, ws, bs in (
            ("cgbq", w_qkv[:, 0:D][:, csl], b_qkv[0:D][csl]),
            ("cgbk", w_qkv[:, D:2 * D][:, csl], b_qkv[D:2 * D][csl]),
            ("cgbv", w_qkv[:, 2 * D:3 * D][:, csl], b_qkv[2 * D:3 * D][csl]),
        ):
            cg = ln1_g @ ws
            cb = bs + ln1_b @ ws
            m[nm] = np.ascontiguousarray(np.stack([cg, cb])).astype(ml_dtypes.bfloat16)
        m["wo2"] = np.ascontiguousarray(w_out[csl, :])
        m["xot"] = np.ascontiguousarray(
            np.asarray(xt[:, c * LQ:(c + 1) * LQ], np.float32))
        in_maps.append(m)
    return in_maps


def kernel(**inputs):
    nc = build()
    in_maps = prep_inputs(inputs)
    res = run_bass_kernel_spmd(nc, in_maps, core_ids=list(range(NCORES)))
    out = np.concatenate([res.results[c]["out_t"] for c in range(NCORES)], axis=1)
    return np.ascontiguousarray(out.T).astype(np.float32)


# revision 7
# speedup vs baseline: 1.1422x; 1.0293x over previous
"""Trainium2 Bass kernel for EnhancedMultiHeadSelfAttention.

Sharding: tensor-parallel attention by heads (core c owns heads 2c, 2c+1 over
ALL 2048 tokens) + sequence-parallel FFN (core c owns tokens 256c..256c+255).
The out-projection partial [2048, 1024] is combined with a single DRAM
ReduceScatter (layout [8, 1024, 256] so the flat-chunk scatter hands each core
its own 256 token columns, feature-major).

Math notes (beyond the baseline's):
 - LN1 is folded into the QKV matmuls: with per-token mean mu and std sd,
   W^T LN(x) = (W diag(g))^T x * r - mu*r*cg + cb  (r = 1/sd, cg = W^T g,
   cb = b + W^T ln1_b).  Dividing by r>0 is free for Q and K (cosine attention
   normalizes them per token), so q' = Wg^T x + cg*(-mu) + cb*sd — one K=2
   rank-1 matmul accumulated into the projection PSUM group. V keeps the same
   rank-1 term and a final per-token r multiply (applied post-transpose where
   tokens sit on partitions).
 - The per-token r for V reaches token-partitions via a tiny K=1 transposing
   matmul (lhsT = r row-slice, rhs = [1,1] ones).
 - softmax needs no max-subtraction; only the key-side lcc bias matters; it is
   applied MULTIPLICATIVELY (exp(s+b) = exp(s)*exp(b)) by scaling V's rows and
   the appended denominator column by host-computed exp(b_k), so the exp
   activation needs no bias and can batch two key-chunks per instruction.
 - FFN weights, h, normed2, and the ReduceScatter payload are bf16 (PSUM
   accumulation stays fp32).
"""

import ml_dtypes
import numpy as np

import concourse.bass as bass
import concourse.tile as tile
from concourse import bacc, mybir
from concourse.bass_utils import run_bass_kernel_spmd

F32 = mybir.dt.float32
F32R = mybir.dt.float32r
BF16 = mybir.dt.bfloat16

L = 2048          # sequence length
D = 1024          # model dim
H = 16            # heads (2 per core)
DH = 64           # head dim
FF = 4096         # ffn hidden
P = 128           # partitions
NCORES = 8
LQ = L // NCORES  # 256 own tokens per core (FFN + output)
DC = D // P       # 8 d-model chunks
FC = FF // P      # 32 ffn chunks
KC = L // P       # 16 key chunks of 128
NBLK = 4          # token blocks of 512
BLK = L // NBLK   # 512

GELU_FUNC = mybir.ActivationFunctionType.Gelu

LN_EPS = 1e-5
NORM_EPS = 1e-12
SCALING = DH ** -0.5
LCC = 0.1


def _mm(nc, out, lhsT, rhs, start, stop):
    assert lhsT.dtype == rhs.dtype and lhsT.dtype in (F32R, BF16), \
        (lhsT.dtype, rhs.dtype)
    nc.tensor.matmul(out, lhsT, rhs, start=start, stop=stop)


def emit(tc):
    nc = tc.nc

    xt = nc.dram_tensor("xt", [D, L], BF16, kind="ExternalInput").ap()
    xot = nc.dram_tensor("xot", [D, LQ], F32R, kind="ExternalInput").ap()
    wq2 = nc.dram_tensor("wq2", [D, P], BF16, kind="ExternalInput").ap()
    wk2 = nc.dram_tensor("wk2", [D, P], BF16, kind="ExternalInput").ap()
    wv2 = nc.dram_tensor("wv2", [D, P], BF16, kind="ExternalInput").ap()
    wo2 = nc.dram_tensor("wo2", [P, D], F32R, kind="ExternalInput").ap()
    wf1 = nc.dram_tensor("wf1", [D, FF], BF16, kind="ExternalInput").ap()
    wf2 = nc.dram_tensor("wf2", [FF, D], BF16, kind="ExternalInput").ap()
    cgbq = nc.dram_tensor("cgbq", [2, P], BF16, kind="ExternalInput").ap()
    cgbk = nc.dram_tensor("cgbk", [2, P], BF16, kind="ExternalInput").ap()
    cgbv = nc.dram_tensor("cgbv", [2, P], BF16, kind="ExternalInput").ap()
    bo = nc.dram_tensor("bo", [P, DC], F32, kind="ExternalInput").ap()
    bf1 = nc.dram_tensor("bf1", [P, FC], F32, kind="ExternalInput").ap()
    bf2 = nc.dram_tensor("bf2", [P, DC], F32, kind="ExternalInput").ap()
    explcc = nc.dram_tensor("explcc", [P, KC], F32, kind="ExternalInput").ap()
    sel2 = nc.dram_tensor("sel2", [P, 2], F32R, kind="ExternalInput").ap()
    selb2 = nc.dram_tensor("selb2", [2, P], F32R, kind="ExternalInput").ap()
    selb2q = nc.dram_tensor("selb2q", [2, P], F32R, kind="ExternalInput").ap()
    ident = nc.dram_tensor("ident", [P, P], F32R, kind="ExternalInput").ap()
    ones1r = nc.dram_tensor("ones1r", [1, P], F32R, kind="ExternalInput").ap()
    ones1f = nc.dram_tensor("ones1f", [1, P], F32, kind="ExternalInput").ap()
    onesc = nc.dram_tensor("onesc", [P, 1], F32R, kind="ExternalInput").ap()
    out_t = nc.dram_tensor("out_t", [D, LQ], F32, kind="ExternalOutput").ap()

    xt3 = xt.rearrange("(c p) t -> p c t", p=P)        # [128, 8, 2048]
    xot3 = xot.rearrange("(c p) t -> p c t", p=P)      # [128, 8, 256]
    wq3 = wq2.rearrange("(c p) n -> p c n", p=P)       # [128, 8, 128]
    wk3 = wk2.rearrange("(c p) n -> p c n", p=P)
    wv3 = wv2.rearrange("(c p) n -> p c n", p=P)
    wf13 = wf1.rearrange("(c p) n -> p c n", p=P)      # [128, 8, 4096]
    wf23 = wf2.rearrange("(c p) n -> p c n", p=P)      # [128, 32, 1024]
    out3 = out_t.rearrange("(c p) t -> p c t", p=P)    # [128, 8, 256]

    # ---- x blocks first: the stats/QKV pipeline is the critical path ----
    xb_pool0 = tc.alloc_tile_pool(name="xb", bufs=NBLK)
    xbs = []
    for b in range(NBLK):
        xb = xb_pool0.tile([P, DC, BLK], BF16, tag="xb", name=f"xb{b}")
        for u in range(2):
            csl = slice(u * DC // 2, (u + 1) * DC // 2)
            nc.sync.dma_start(xb[:, csl, :],
                              xt3[:, csl, b * BLK:(b + 1) * BLK])
        xbs.append(xb)

    # ---- persistent small constants -------------------------------------
    singles = tc.alloc_tile_pool(name="singles", bufs=1)
    ones_1x128 = singles.tile([1, P], F32R)
    nc.sync.dma_start(ones_1x128, ones1r)
    ones1f_sb = singles.tile([1, P], F32)
    nc.sync.dma_start(ones1f_sb, ones1f)
    ones_col = singles.tile([P, 1], F32R)
    nc.sync.dma_start(ones_col, onesc)
    ones_col_bf = singles.tile([P, 1], BF16)
    nc.gpsimd.dma_start(ones_col_bf, onesc)
    sel2_sb = singles.tile([P, 2], F32R)
    nc.sync.dma_start(sel2_sb, sel2)
    selb2_sb = singles.tile([2, P], F32R)
    nc.sync.dma_start(selb2_sb, selb2)
    selb2q_sb = singles.tile([2, P], F32R)
    nc.sync.dma_start(selb2q_sb, selb2q)
    ident_sb = singles.tile([P, P], F32R)
    nc.sync.dma_start(ident_sb, ident)
    cgbq_sb = singles.tile([2, P], BF16)
    nc.sync.dma_start(cgbq_sb, cgbq)
    cgbk_sb = singles.tile([2, P], BF16)
    nc.sync.dma_start(cgbk_sb, cgbk)
    cgbv_sb = singles.tile([2, P], BF16)
    nc.sync.dma_start(cgbv_sb, cgbv)
    bo_sb = singles.tile([P, DC], F32)
    nc.sync.dma_start(bo_sb, bo)
    bf1_sb = singles.tile([P, FC], F32)
    nc.sync.dma_start(bf1_sb, bf1)
    bf2_sb = singles.tile([P, DC], F32)
    nc.sync.dma_start(bf2_sb, bf2)
    explcc_sb = singles.tile([P, KC], F32)
    nc.sync.dma_start(explcc_sb, explcc)
    wo2_sb = singles.tile([P, DC, P], F32R)
    nc.sync.dma_start(wo2_sb, wo2.rearrange("p (c n) -> p c n", n=P))
    wq_sb = singles.tile([P, DC, P], BF16)
    nc.sync.dma_start(wq_sb, wq3)
    wk_sb = singles.tile([P, DC, P], BF16)
    nc.sync.dma_start(wk_sb, wk3)
    wv_sb = singles.tile([P, DC, P], BF16)
    nc.sync.dma_start(wv_sb, wv3)
    eps_sb = singles.tile([1, 1], F32)
    nc.vector.memset(eps_sb, LN_EPS)

    # persistent activation tiles (released before FFN where possible)
    qk_pool = tc.alloc_tile_pool(name="qk", bufs=1)
    q_t = qk_pool.tile([P, L], F32R)     # [2 heads x 64 dims, tokens]
    k_t = qk_pool.tile([P, L], F32R)
    v_tm = qk_pool.tile([P, KC, 2 * (DH + 1)], F32R)  # [keys, kc, (d+1)*2h]

    # DRAM scratch: out-proj partial, one buffer per own-token half so the
    # first ReduceScatter can fire while the second half's attention runs
    LH = LQ // 2
    podram_pool = tc.alloc_tile_pool(name="podram", bufs=1, space="DRAM")
    po_dram = podram_pool.tile([2, NCORES, D, LH], BF16)
    rs_pool = tc.alloc_tile_pool(name="rsdram", bufs=1, space="DRAM")
    rs_dram = rs_pool.tile([2, D, LH], BF16)

    # =====================================================================
    # Phase 1: stats + QKV projections for the core's 2 heads, all tokens
    # =====================================================================
    with (
        tc.tile_pool(name="sq", bufs=3) as sq_pool,
        tc.tile_pool(name="smalls", bufs=3) as smalls,
        tc.tile_pool(name="vstage", bufs=3) as vstage_pool,
        tc.tile_pool(name="ps_stat", bufs=1, space="PSUM") as ps_stat,
        tc.tile_pool(name="ps_mm", bufs=3, space="PSUM") as ps_mm,
        tc.tile_pool(name="ps_nrm", bufs=1, space="PSUM") as ps_nrm,
        tc.tile_pool(name="ps_vt", bufs=1, space="PSUM") as ps_vt_pool,
    ):
        for b in range(NBLK):
            sl = slice(b * BLK, (b + 1) * BLK)
            xb = xbs[b]
            # token stats: sums (row 0) and sum-of-squares (row 1) via
            # ones-column matmuls into disjoint partition rows of one bank
            sums = ps_stat.tile([1, BLK], F32, tag="sums")
            sumsq = ps_stat.tile([1, BLK], F32, tag="sumsq")
            xsq = sq_pool.tile([P, DC, BLK], F32R, tag="xsq")
            nc.scalar.square(xsq[:, 0:DC // 2, :], xb[:, 0:DC // 2, :])
            nc.scalar.square(xsq[:, DC // 2:, :], xb[:, DC // 2:, :])
            for c in range(DC):
                _mm(nc, sums, ones_col_bf, xb[:, c, :], c == 0, c == DC - 1)
                _mm(nc, sumsq, ones_col, xsq[:, c, :], c == 0, c == DC - 1)
            # rhs2 = [-mu; sd] for the rank-1 LN fold; r = 1/sd for V
            mu = smalls.tile([1, BLK], F32, tag="mu")
            nc.vector.tensor_scalar_mul(mu, sums, 1.0 / D)
            ex2 = smalls.tile([1, BLK], F32, tag="ex2")
            nc.vector.tensor_scalar_mul(ex2, sumsq, 1.0 / D)
            var = smalls.tile([1, BLK], F32, tag="var")
            nc.vector.tensor_mul(var, mu, mu)
            nc.vector.tensor_sub(var, ex2, var)
            rhs2 = smalls.tile([2, BLK], BF16, tag="rhs2")
            sd0 = smalls.tile([1, BLK], F32, tag="sd0")
            nc.scalar.activation(sd0, var,
                                 func=mybir.ActivationFunctionType.Sqrt,
                                 bias=eps_sb, scale=1.0)
            with nc.allow_low_precision(reason="f32r matmul operand"):
                nc.vector.tensor_scalar_mul(rhs2[0:1, :], mu, -1.0)
            nc.gpsimd.dma_start(rhs2[1:2, :], sd0)
            r_row = smalls.tile([1, BLK], F32, tag="rrow")
            nc.vector.reciprocal(r_row, sd0)

            # Q / K with cosine normalization folded
            for (wsb, cgb, dst, scaled) in (
                (wq_sb, cgbq_sb, q_t, True),
                (wk_sb, cgbk_sb, k_t, False),
            ):
                ps = ps_mm.tile([P, BLK], F32, tag="mm")
                for c in range(DC):
                    _mm(nc, ps, wsb[:, c, :], xb[:, c, :], c == 0, False)
                _mm(nc, ps, cgb, rhs2, False, True)
                psq = sq_pool.tile([P, BLK], F32R, tag="psq")
                nc.scalar.square(psq, ps)
                nsq = ps_nrm.tile([2, BLK], F32, tag="aux")
                _mm(nc, nsq, sel2_sb, psq, True, True)
                sdq = smalls.tile([2, BLK], F32, tag="sdq")
                nc.scalar.activation(sdq, nsq,
                                     func=mybir.ActivationFunctionType.Sqrt,
                                     bias=0.0, scale=1.0)
                rec = smalls.tile([2, BLK], F32R, tag="rec")
                with nc.allow_low_precision(reason="f32r matmul operand"):
                    nc.vector.reciprocal(rec, sdq)
                rbc = ps_nrm.tile([P, BLK], F32, tag="rbc")
                _mm(nc, rbc, selb2q_sb if scaled else selb2_sb, rec,
                    True, True)
                rbc_sb = smalls.tile([P, BLK], F32, tag="rbcsb")
                nc.vector.tensor_copy(rbc_sb, rbc)
                nc.vector.tensor_mul(dst[:, sl], ps, rbc_sb)

            # V: rank-1 fold, PE transpose to token-major, then r multiply
            ps = ps_mm.tile([P, BLK], F32, tag="mm")
            for c in range(DC):
                _mm(nc, ps, wv_sb[:, c, :], xb[:, c, :], c == 0, False)
            _mm(nc, ps, cgbv_sb, rhs2, False, True)
            vstage = vstage_pool.tile([P, BLK], F32R, tag="vstage")
            nc.vector.tensor_copy(vstage, ps)
            for t in range(4):
                kc = b * 4 + t
                tsl = slice(t * P, (t + 1) * P)
                vt = ps_vt_pool.tile([P, BLK], F32R, tag="vt")
                nc.tensor.matmul(vt[:, 0:P], vstage[:, tsl], ident_sb,
                                 is_transpose=True, start=True, stop=True)
                rtm = ps_nrm.tile([P, 1], F32, tag="aux")
                nc.tensor.matmul(rtm, r_row[:, tsl], ones1f_sb[:, 0:1],
                                 start=True, stop=True)
                # fold exp(key-side lcc bias) into V rows and the ones column
                rtme = smalls.tile([P, 1], F32, tag="rtme")
                nc.vector.tensor_mul(rtme, explcc_sb[:, kc:kc + 1], rtm)
                nc.vector.tensor_scalar_mul(v_tm[:, kc, 0:DH], vt[:, 0:DH],
                                            rtme)
                nc.vector.tensor_scalar_mul(v_tm[:, kc, DH + 1:2 * DH + 1],
                                            vt[:, DH:2 * DH], rtme)
            nc.gpsimd.tensor_copy(
                v_tm[:, b * 4:(b + 1) * 4, DH:DH + 1],
                explcc_sb[:, b * 4:(b + 1) * 4].unsqueeze(2))
            nc.gpsimd.tensor_copy(
                v_tm[:, b * 4:(b + 1) * 4, 2 * DH + 1:],
                explcc_sb[:, b * 4:(b + 1) * 4].unsqueeze(2))

    # =====================================================================
    # Phase 2: attention for 2 heads over all queries + out-proj partials
    # (first half of wf1 prefetches in the background; rest streams in ff1)
    # =====================================================================
    wf1_pool = tc.alloc_tile_pool(name="wf1sb", bufs=1)
    wf1_sb = wf1_pool.tile([P, DC, FF], BF16)
    for sl4 in range(4):
        nc.sync.dma_start(wf1_sb[:, :, sl4 * FF // 4:(sl4 + 1) * FF // 4],
                          wf13[:, :, sl4 * FF // 4:(sl4 + 1) * FF // 4])

    with (
        tc.tile_pool(name="eh", bufs=4) as eh_pool,
        tc.tile_pool(name="apair", bufs=3) as apair_pool,
        tc.tile_pool(name="rsc", bufs=3) as rsc_pool,
        tc.tile_pool(name="postage", bufs=2) as postage_pool,
        tc.tile_pool(name="ps_sc", bufs=2, space="PSUM") as ps_sc,
        tc.tile_pool(name="ps_acc", bufs=2, space="PSUM") as ps_acc,
        tc.tile_pool(name="ps_rbc", bufs=1, space="PSUM") as ps_rbc,
        tc.tile_pool(name="ps_op", bufs=1, space="PSUM") as ps_op,
    ):
        # query sets: half z of every 256-token chunk, chunk groups of 4.
        # set (h, g) covers queries {256c + h*128 + t : c in 4g..4g+3}.
        def qview(src, j, g, h):
            return src[j * DH:(j + 1) * DH, :].rearrange(
                "p (c z t) -> p c z t", z=2, t=LH)[:, 4 * g:4 * g + 4, h, :]

        for h in range(2):
            for g in range(2):
                apair = apair_pool.tile([P, BLK], F32R, tag="apair")
                for j in range(2):
                    acc = ps_acc.tile([DH + 1, BLK], F32, tag="acc")
                    for kc2 in range(KC // 2):
                        ps = ps_sc.tile([P, 2, BLK], F32, tag="sc")
                        eh = eh_pool.tile([P, 2, BLK], F32R, tag="eh")
                        for u in range(2):
                            kc = 2 * kc2 + u
                            _mm(nc, ps[:, u, :],
                                k_t[j * DH:(j + 1) * DH, kc * P:(kc + 1) * P],
                                qview(q_t, j, g, h), True, True)
                        nc.scalar.activation(
                            eh, ps, func=mybir.ActivationFunctionType.Exp,
                            bias=0.0, scale=1.0)
                        for u in range(2):
                            kc = 2 * kc2 + u
                            _mm(nc, acc,
                                v_tm[:, kc, j * (DH + 1):(j + 1) * (DH + 1)],
                                eh[:, u, :], kc == 0, kc == KC - 1)
                    recip = rsc_pool.tile([1, BLK], F32R, tag="recip")
                    with nc.allow_low_precision(reason="f32r matmul operand"):
                        nc.vector.reciprocal(recip, acc[DH:DH + 1, :])
                    rbc = ps_rbc.tile([DH, BLK], F32, tag="rbc")
                    _mm(nc, rbc, ones_1x128[:, 0:DH], recip, True, True)
                    rbc_sb = rsc_pool.tile([DH, BLK], F32, tag="rbcsb")
                    nc.vector.tensor_copy(rbc_sb, rbc)
                    nc.vector.tensor_mul(apair[j * DH:(j + 1) * DH, :],
                                         acc[0:DH, :], rbc_sb)
                # out-projection partial for this query set
                postage = postage_pool.tile([P, DC, BLK], BF16, tag="po")
                for o in range(DC):
                    pso = ps_op.tile([P, BLK], F32, tag="op")
                    _mm(nc, pso, wo2_sb[:, o, :], apair, True, True)
                    nc.vector.tensor_copy(postage[:, o, :], pso)
                for o in range(DC):
                    nc.sync.dma_start(
                        po_dram[h, 4 * g:4 * g + 4, o * P:(o + 1) * P, :]
                        .rearrange("s p t -> p s t"),
                        postage[:, o, :].rearrange("p (s t) -> p s t", t=LH))
            if h == 0:
                nc.gpsimd.collective_compute(
                    "ReduceScatter", mybir.AluOpType.add,
                    replica_groups=[list(range(NCORES))],
                    ins=[po_dram[0]], outs=[rs_dram[0]])

    # =====================================================================
    # Phase 3/4 pipelined by own-token half: residual+LN2+ff1 for half A
    # overlap the second ReduceScatter; ff2 runs monolithic at the end.
    # =====================================================================
    with (
        tc.tile_pool(name="x2p", bufs=1) as x2_pool,
        tc.tile_pool(name="ffsq", bufs=2) as ffsq_pool,
        tc.tile_pool(name="ffsm", bufs=2) as ffsm,
        tc.tile_pool(name="ht", bufs=1) as ht_pool,
        tc.tile_pool(name="wf2s", bufs=8) as wf2s,
        tc.tile_pool(name="outsb", bufs=2) as outsb_pool,
    ):
        x2 = x2_pool.tile([P, DC, LQ], F32R)
        xo2 = x2_pool.tile([P, DC, LQ], F32R)
        nc.sync.dma_start(xo2, xot3)
        normed2 = x2_pool.tile([P, DC, LQ], BF16)
        h_t = ht_pool.tile([P, FC, LQ], BF16)
        ps_mm3 = tc.alloc_tile_pool(name="ps_mm3", bufs=4, space="PSUM")

        def half_ln_ff1(h):
            hsl = slice(h * LH, (h + 1) * LH)
            rs_sb = x2_pool.tile([P, DC, LH], BF16, tag="rssb", bufs=2)
            nc.sync.dma_start(
                rs_sb, rs_dram[h].rearrange("(c p) t -> p c t", p=P))
            for o in range(DC):
                nc.vector.tensor_scalar_add(x2[:, o, hsl], rs_sb[:, o, :],
                                            bo_sb[:, o:o + 1])
                nc.vector.tensor_add(x2[:, o, hsl], x2[:, o, hsl],
                                     xo2[:, o, hsl])
            with (
                tc.tile_pool(name=f"ps_st{h}", bufs=1, space="PSUM") as ps3,
                tc.tile_pool(name=f"ps_cf{h}", bufs=2, space="PSUM") as psc3,
            ):
                sums = ps3.tile([1, LH], F32, tag="sums")
                sumsq = ps3.tile([1, LH], F32, tag="sumsq")
                # stats via bf16 operands: fp32r matmuls at N=128 run at 1/4
                # rate, bf16 at full rate; the normalize itself stays fp32
                xsq = ffsq_pool.tile([P, DC, LH], BF16, tag="xsq")
                nc.scalar.square(xsq, x2[:, :, hsl])
                x2b = ffsq_pool.tile([P, DC, LH], BF16, tag="x2b")
                nc.vector.tensor_copy(x2b, x2[:, :, hsl])
                for c in range(DC):
                    _mm(nc, sums, ones_col_bf, x2b[:, c, :], c == 0, c == DC - 1)
                    _mm(nc, sumsq, ones_col_bf, xsq[:, c, :], c == 0, c == DC - 1)
                mu = ffsm.tile([1, LH], F32, tag="mu")
                nc.vector.tensor_scalar_mul(mu, sums, 1.0 / D)
                ex2 = ffsm.tile([1, LH], F32, tag="ex2")
                nc.vector.tensor_scalar_mul(ex2, sumsq, 1.0 / D)
                var = ffsm.tile([1, LH], F32, tag="var")
                nc.vector.tensor_mul(var, mu, mu)
                nc.vector.tensor_sub(var, ex2, var)
                sd = ffsm.tile([1, LH], F32, tag="sd")
                nc.scalar.activation(sd, var,
                                     func=mybir.ActivationFunctionType.Sqrt,
                                     bias=eps_sb, scale=1.0)
                rstd = ffsm.tile([1, LH], F32R, tag="rstd")
                with nc.allow_low_precision(reason="f32r matmul operand"):
                    nc.vector.reciprocal(rstd, sd)
                shift = ffsm.tile([1, LH], F32R, tag="shift")
                nc.vector.tensor_mul(shift, mu, rstd)
                nc.vector.tensor_scalar_mul(shift, shift, -1.0)
                rstd_bc = psc3.tile([P, LH], F32, tag="coef")
                shift_bc = psc3.tile([P, LH], F32, tag="coef")
                _mm(nc, rstd_bc, ones_1x128, rstd, True, True)
                _mm(nc, shift_bc, ones_1x128, shift, True, True)
                n2h = normed2[:, :, hsl]
                rb = rstd_bc.unsqueeze(1).to_broadcast(n2h.shape)
                sb = shift_bc.unsqueeze(1).to_broadcast(n2h.shape)
                nc.vector.tensor_mul(n2h, x2[:, :, hsl], rb)
                nc.vector.tensor_add(n2h, n2h, sb)
            for f in range(FC):
                ps = ps_mm3.tile([P, LH], F32, tag="mm")
                for c in range(DC):
                    _mm(nc, ps, wf1_sb[:, c, f * P:(f + 1) * P],
                        normed2[:, c, hsl], c == 0, c == DC - 1)
                nc.scalar.activation(h_t[:, f, hsl], ps, func=GELU_FUNC,
                                     bias=bf1_sb[:, f:f + 1], scale=1.0)

        nc.gpsimd.collective_compute(
            "ReduceScatter", mybir.AluOpType.add,
            replica_groups=[list(range(NCORES))],
            ins=[po_dram[1]], outs=[rs_dram[1]])
        half_ln_ff1(0)
        half_ln_ff1(1)

        # ff2: two 4-output passes (PSUM accumulation groups are
        # bank-granular, so only 4 + ff1's 3 banks fit); wf2 for the second
        # pass prefetches during the first so pass 2 is pure PE
        ps_ff2 = tc.alloc_tile_pool(name="ps_ff2", bufs=4, space="PSUM")
        wf24 = wf23.rearrange("p c (g n) -> p c g n", g=2)  # [128,32,2,512]
        for g in range(2):
            accs = [ps_ff2.tile([P, LQ], F32, tag="ff2acc",
                                name=f"ff2acc_{g}_{i}") for i in range(4)]
            for f2 in range(FC // 2):
                wf2m = wf2s.tile([P, 2, 4 * P], BF16, tag="wf2")
                nc.sync.dma_start(wf2m, wf24[:, 2 * f2:2 * f2 + 2, g, :])
                for r in range(2):
                    f = 2 * f2 + r
                    for i in range(4):
                        _mm(nc, accs[i], wf2m[:, r, i * P:(i + 1) * P],
                            h_t[:, f, :], f == 0, f == FC - 1)
            for i in range(4):
                mcol = g * 4 + i
                osb = outsb_pool.tile([P, LQ], F32, tag="osb")
                nc.vector.tensor_scalar_add(osb, accs[i],
                                            bf2_sb[:, mcol:mcol + 1])
                nc.vector.tensor_add(osb, osb, x2[:, mcol, :])
                nc.sync.dma_start(out3[:, mcol, :], osb)
        ps_ff2.release()
        ps_mm3.release()

    wf1_pool.release()
    rs_pool.release()
    podram_pool.release()
    qk_pool.release()
    singles.release()
    xb_pool0.release()


_CACHED = None


def build():
    global _CACHED
    if _CACHED is None:
        nc = bacc.Bacc("TRN2", target_bir_lowering=False, debug=False,
                       num_devices=NCORES)
        with tile.TileContext(nc) as tc:
            emit(tc)
        nc.compile()
        _CACHED = nc
    return _CACHED


def prep_inputs(inputs):
    """Host-side preprocessing: transposes, slices, LN folds."""
    f = np.float32
    x = np.asarray(inputs["x"], f)
    lcc = np.asarray(inputs["lcc_values"], f)
    w_qkv = np.asarray(inputs["w_qkv"], f)
    b_qkv = np.asarray(inputs["b_qkv"], f)
    w_out = np.asarray(inputs["w_out"], f)
    ln1_g = np.asarray(inputs["ln1_g"], f)
    ln1_b = np.asarray(inputs["ln1_b"], f)
    ln2_g = np.asarray(inputs["ln2_g"], f)
    ln2_b = np.asarray(inputs["ln2_b"], f)
    w_ff1 = np.asarray(inputs["w_ff1"], f)
    b_ff1 = np.asarray(inputs["b_ff1"], f)

    def chunked(b):  # [D] -> [128, DC] with chunk c in column c
        return np.ascontiguousarray(b.reshape(-1, P).T)

    xt = np.ascontiguousarray(x.T).astype(ml_dtypes.bfloat16)
    sel2_m = np.zeros((P, 2), f)
    sel2_m[0:DH, 0] = 1.0
    sel2_m[DH:P, 1] = 1.0

    shared = {
        "xt": xt,
        "wf1": np.ascontiguousarray(ln2_g[:, None] * w_ff1).astype(ml_dtypes.bfloat16),
        "wf2": np.ascontiguousarray(np.asarray(inputs["w_ff2"], f)).astype(ml_dtypes.bfloat16),
        "bo": chunked(np.asarray(inputs["b_out"], f)),
        "bf1": chunked(b_ff1 + ln2_b @ w_ff1),
        "bf2": chunked(np.asarray(inputs["b_ff2"], f)),
        "explcc": np.ascontiguousarray(np.exp(lcc * (0.5 * LCC)).reshape(KC, P).T),
        "sel2": sel2_m,
        "selb2": np.ascontiguousarray(sel2_m.T),
        "selb2q": np.ascontiguousarray(sel2_m.T * (DH ** -0.5)),
        "ident": np.eye(P, dtype=f),
        "ones1r": np.ones((1, P), f),
        "ones1f": np.ones((1, P), f),
        "onesc": np.ones((P, 1), f),
    }
    in_maps = []
    for c in range(NCORES):
        m = dict(shared)
        csl = slice(c * P, (c + 1) * P)
        wq_s = ln1_g[:, None] * w_qkv[:, 0:D][:, csl]
        wk_s = ln1_g[:, None] * w_qkv[:, D:2 * D][:, csl]
        wv_s = ln1_g[:, None] * w_qkv[:, 2 * D:3 * D][:, csl]
        m["wq2"] = np.ascontiguousarray(wq_s).astype(ml_dtypes.bfloat16)
        m["wk2"] = np.ascontiguousarray(wk_s).astype(ml_dtypes.bfloat16)
        m["wv2"] = np.ascontiguousarray(wv_s).astype(ml_dtypes.bfloat16)
        for nm, ws, bs in (
            ("cgbq", w_qkv[:, 0:D][:, csl], b_qkv[0:D][csl]),
            ("cgbk", w_qkv[:, D:2 * D][:, csl], b_qkv[D:2 * D][csl]),
            ("cgbv", w_qkv[:, 2 * D:3 * D][:, csl], b_qkv[2 * D:3 * D][csl]),
        ):
            cg = ln1_g @ ws
            cb = bs + ln1_b @ ws
            m[nm] = np.ascontiguousarray(np.stack([cg, cb])).astype(ml_dtypes.bfloat16)
        m["wo2"] = np.ascontiguousarray(w_out[csl, :])
        m["xot"] = np.ascontiguousarray(
            np.asarray(xt[:, c * LQ:(c + 1) * LQ], np.float32))
        in_maps.append(m)
    return in_maps


def kernel(**inputs):
    nc = build()
    in_maps = prep_inputs(inputs)
    res = run_bass_kernel_spmd(nc, in_maps, core_ids=list(range(NCORES)))
    out = np.concatenate([res.results[c]["out_t"] for c in range(NCORES)], axis=1)
    return np.ascontiguousarray(out.T).astype(np.float32)
